# revision 1
# baseline (speedup 1.0000x reference)
"""Trainium2 Bass kernel for nn_CrossAttention (B=2, C=512, N=M=2048, H=8).

Sharding: batch*heads = 16 (b,h) pairs across 8 cores, 2 heads per core.
Cores 0-3 handle batch 0 (heads 0..7 in pairs), cores 4-7 batch 1.

Per-core math (all matmuls fp32r = tf32-like, full PE rate at free>=256):
  qT[d,n] = (Wq_cols * SCALE).T @ x_b          (2 heads packed on partitions)
  kT[d,m] = Wk_cols.T @ y_b
  vT[d,m] = (Wv_cols * (1+lw)).T @ y_b   -> PE-transpose -> v2[m, d|1] tiles
  S^T[m,n] = kT_h.T-slices @ qT_h        (row-packed K=64 pairs per head)
  P = exp(S^T)                            (ScalarE, streaming blocks)
  [attnT | den] = [v2_h | ones].T @ P     (M=65 ones-augmented, accum over m)
  attnT_norm = attnT * (1/den)            (gpsimd partition-broadcast + DVE)
  outT_partial[c,n] = Wp_rows.T @ attnT_norm

The depthwise conv (ksize=1) folds into Wv scaling + a host-side output bias
(bias' = bp + lb @ Wp, exact because softmax rows sum to 1).
Host sums the 4 per-batch partials and adds bias'.
"""

import os
import sys
import numpy as np
from contextlib import ExitStack

for _p in ("/root/.axon_site", "/root/.axon_site/_ro/trn_rl_repo",
           "/root/.axon_site/_ro/pypackages", "/opt/trn_rl_repo"):
    if os.path.isdir(_p) and _p not in sys.path:
        sys.path.append(_p)

B, C, N, M, H = 2, 512, 2048, 2048, 8
HD = C // H
SCALE = HD ** -0.5
NCORES = 8

_NC = None
LAST_RUN = None


def to_fp32r(x: np.ndarray) -> np.ndarray:
    """Round fp32 to the 20-bit (1s/8e/11m) fp32r grid, round-to-nearest-even."""
    b = np.ascontiguousarray(x, np.float32).view(np.uint32).astype(np.uint64)
    rb = (b >> 12) & 1
    b = (b + 0x7FF + rb) & 0xFFFFF000
    return b.astype(np.uint32).view(np.float32)


def _build_program(reps=1):
    from concourse import bacc
    import concourse.tile as tile
    import concourse.mybir as mybir
    from concourse.masks import make_identity

    F32 = mybir.dt.float32
    F32R = mybir.dt.float32r
    EXP = mybir.ActivationFunctionType.Exp
    MULT = mybir.AluOpType.mult

    nc = bacc.Bacc("TRN2", target_bir_lowering=False, debug=False,
                   num_devices=NCORES)

    xr = nc.dram_tensor("xr", [C, N], F32R, kind="ExternalInput").ap()
    yr = nc.dram_tensor("yr", [C, M], F32R, kind="ExternalInput").ap()
    wq_d = nc.dram_tensor("wq", [C, 128], F32R, kind="ExternalInput").ap()
    wk_d = nc.dram_tensor("wk", [C, 128], F32R, kind="ExternalInput").ap()
    wv_d = nc.dram_tensor("wv", [C, 128], F32R, kind="ExternalInput").ap()
    wp_d = nc.dram_tensor("wp", [128, C], F32R, kind="ExternalInput").ap()
    ones_d = nc.dram_tensor("ones_d", [128, 1], F32R, kind="ExternalInput").ap()
    outT = nc.dram_tensor("outT", [C, N], F32, kind="ExternalOutput").ap()

    with tile.TileContext(nc) as tc, ExitStack() as ctx:
        sb = ctx.enter_context(tc.tile_pool(name="sb", bufs=1))
        ppool = ctx.enter_context(tc.tile_pool(name="ppool", bufs=4))
        npool = ctx.enter_context(tc.tile_pool(name="npool", bufs=2))
        spool = ctx.enter_context(tc.tile_pool(name="spool", bufs=2))
        # PSUM budget (8 banks): psA "blk" 3x[128,1024] = 6 banks (score
        # ring, also proj accumulators / transposes / outproj transients);
        # psB "acc" 2x[65,512] = 2 banks (attn accumulators). Ring depth 3
        # decouples PE from ScalarE's exp stream.
        psA = ctx.enter_context(tc.tile_pool(name="psA", bufs=3, space="PSUM"))
        psB = ctx.enter_context(tc.tile_pool(name="psB", bufs=2, space="PSUM"))

        # ---- constants / weights ----
        ident = sb.tile([128, 128], F32, tag="ident")
        make_identity(nc, ident)
        ones_sb = sb.tile([128, 1], F32R, tag="ones_sb")
        nc.sync.dma_start(out=ones_sb, in_=ones_d)
        # warm the exp table while DMAs stream
        warm = sb.tile([1, 32], F32, tag="warm")
        nc.scalar.activation(warm, ident[0:1, 0:32], EXP)
        # warm the PE clock (HAM) with dummy matmuls so the first
        # projections run at 2.4GHz; transposes don't count as PE-busy.
        psw = psB.tile([128, 128], F32, tag="acc", name="psw")
        for _ in range(8):
            nc.tensor.matmul(psw, ident, ident, start=True, stop=True)
        warm2 = sb.tile([128, 128], F32, tag="warm2")
        nc.vector.tensor_copy(warm2, psw)

        wk_sb = sb.tile([128, 4, 128], F32R, tag="wk_sb")
        wv_sb = sb.tile([128, 4, 128], F32R, tag="wv_sb")
        wq_sb = sb.tile([128, 4, 128], F32R, tag="wq_sb")
        wp_sb = sb.tile([128, C], F32R, tag="wp_sb")

        for rep in range(reps):
            r = f"r{rep}_" if reps > 1 else ""

            # ---- column-sliced input loads on the sync-engine HWDGE ----
            y_sb = [sb.tile([128, M], F32R, tag=f"y_sb{k}", name=f"{r}y_sb{k}")
                    for k in range(4)]
            x_sb = [sb.tile([128, N], F32R, tag=f"x_sb{k}", name=f"{r}x_sb{k}")
                    for k in range(4)]
            # DMA order = consumption order: y j0/j1 gate the prologue
            # projections, x j0 gates qT j0, the rest streams under the
            # main loop (j2/j3 projections are woven into n-chunk 0).
            def load_slices(dst_tiles, src, j):
                js = slice(j * 512, (j + 1) * 512)
                for k in range(4):
                    nc.sync.dma_start(
                        out=dst_tiles[k][:, js],
                        in_=src[k * 128:(k + 1) * 128, js])

            if rep == 0:
                nc.sync.dma_start(
                    out=wk_sb, in_=wk_d.rearrange("(kc p) m -> p kc m", p=128))
            load_slices(y_sb, yr, 0)
            if rep == 0:
                nc.sync.dma_start(
                    out=wv_sb, in_=wv_d.rearrange("(kc p) m -> p kc m", p=128))
            load_slices(x_sb, xr, 0)
            if rep == 0:
                nc.sync.dma_start(
                    out=wq_sb, in_=wq_d.rearrange("(kc p) m -> p kc m", p=128))
            load_slices(y_sb, yr, 1)
            load_slices(y_sb, yr, 2)
            load_slices(y_sb, yr, 3)
            if rep == 0:
                nc.sync.dma_start(out=wp_sb, in_=wp_d)
            load_slices(x_sb, xr, 1)
            load_slices(x_sb, xr, 2)
            load_slices(x_sb, xr, 3)

            kT = sb.tile([128, M], F32R, tag="kT", name=f"{r}kT")
            vT = sb.tile([128, M], F32, tag="vT", name=f"{r}vT")
            qT = sb.tile([128, N], F32R, tag="qT", name=f"{r}qT")
            v2a = [None] * 16
            v2b = [None] * 16

            def proj_half(ps_holder, dst, w_sb, src, j, half, name):
                if half == 0:
                    ps_holder[name] = psA.tile([128, 512], F32, tag="blk",
                                               name=name)
                ps = ps_holder[name]
                for kc in (0, 1) if half == 0 else (2, 3):
                    nc.tensor.matmul(ps, w_sb[:, kc, :],
                                     src[kc][:, j * 512:(j + 1) * 512],
                                     start=(kc == 0), stop=(kc == 3))
                if half == 1:
                    nc.vector.tensor_copy(dst[:, j * 512:(j + 1) * 512], ps)

            def transpose_quad(m0):
                # 4 transposes share one PSUM ring slot (4 col-slices)
                t = psA.tile([128, 512], F32, tag="blk", name=f"{r}pst{m0}")
                for i in range(4):
                    m = m0 + i
                    nc.tensor.transpose(t[:, i * 128:(i + 1) * 128],
                                        vT[:, m * 128:(m + 1) * 128], ident)
                for i in range(4):
                    m = m0 + i
                    c = i * 128
                    a_ = sb.tile([128, 65], F32R, tag=f"v2a{m}",
                                 name=f"{r}v2a{m}")
                    nc.vector.tensor_copy(a_[:, 0:64], t[:, c:c + 64])
                    nc.vector.tensor_copy(a_[:, 64:65], ones_sb)
                    b_ = sb.tile([128, 65], F32R, tag=f"v2b{m}",
                                 name=f"{r}v2b{m}")
                    nc.vector.tensor_copy(b_[:, 0:64], t[:, c + 64:c + 128])
                    nc.vector.tensor_copy(b_[:, 64:65], ones_sb)
                    v2a[m] = a_
                    v2b[m] = b_

            hold = {}

            # ---- prologue: only the j0 chain gates the main loop ----
            for half in (0, 1):
                proj_half(hold, kT, wk_sb, y_sb, 0, half, f"{r}psk0")
            for half in (0, 1):
                proj_half(hold, vT, wv_sb, y_sb, 0, half, f"{r}psv0")
            transpose_quad(0)
            for half in (0, 1):
                proj_half(hold, qT, wq_sb, x_sb, 0, half, f"{r}psq0")

            # fill task groups: one group per m-step, woven between score
            # blocks so the PE finishes late projections without starving
            # ScalarE and without blocking the PSUM ring on late DMAs.
            def P(dst, w, src, j, half, name):
                return lambda: proj_half(hold, dst, w, src, j, half, name)

            fills = {
                0: [[P(kT, wk_sb, y_sb, 1, 0, f"{r}psk1"),
                     P(kT, wk_sb, y_sb, 1, 1, f"{r}psk1")],
                    [P(vT, wv_sb, y_sb, 1, 0, f"{r}psv1"),
                     P(vT, wv_sb, y_sb, 1, 1, f"{r}psv1")],
                    [lambda: transpose_quad(4)],
                    [P(kT, wk_sb, y_sb, 2, 0, f"{r}psk2")],
                    [P(kT, wk_sb, y_sb, 2, 1, f"{r}psk2")],
                    [P(vT, wv_sb, y_sb, 2, 0, f"{r}psv2")],
                    [P(vT, wv_sb, y_sb, 2, 1, f"{r}psv2")],
                    [lambda: transpose_quad(8)],
                    [P(kT, wk_sb, y_sb, 3, 0, f"{r}psk3")],
                    [P(kT, wk_sb, y_sb, 3, 1, f"{r}psk3")],
                    [P(vT, wv_sb, y_sb, 3, 0, f"{r}psv3")],
                    [P(vT, wv_sb, y_sb, 3, 1, f"{r}psv3")],
                    [lambda: transpose_quad(12)],
                    [P(qT, wq_sb, x_sb, 1, 0, f"{r}psq1"),
                     P(qT, wq_sb, x_sb, 1, 1, f"{r}psq1")]],
                1: [[P(qT, wq_sb, x_sb, 2, 0, f"{r}psq2"),
                     P(qT, wq_sb, x_sb, 2, 1, f"{r}psq2")]],
                2: [[P(qT, wq_sb, x_sb, 3, 0, f"{r}psq3"),
                     P(qT, wq_sb, x_sb, 3, 1, f"{r}psq3")]],
                3: [],
            }

            # ---- attention main loop over the global block stream, with
            # attnout lagging one block behind scores/exp so the PE never
            # serializes attnout(n,15) -> scores(n+1,0) at chunk boundaries.
            ah = {}
            pending_out = None   # (n, nrm) awaiting output projection
            prev = None          # (n, m, P, ah0, ah1) awaiting attnout

            def emit_outproj(po_n, po_nrm, cc):
                po = psA.tile([128, 512], F32, tag="blk",
                              name=f"{r}po{po_n}_{cc}")
                nc.tensor.matmul(po, wp_sb[:, cc * 128:(cc + 1) * 128],
                                 po_nrm, start=True, stop=True)
                so = npool.tile([128, 512], F32, tag="so",
                                name=f"{r}so{po_n}_{cc}")
                nc.vector.tensor_copy(so, po)
                nc.sync.dma_start(
                    out=outT[cc * 128:(cc + 1) * 128,
                             po_n * 512:(po_n + 1) * 512],
                    in_=so)

            def emit_attnout(pn, pm, pP, pah0, pah1):
                nonlocal pending_out
                nc.tensor.matmul(pah0, v2a[pm], pP[:, 0:512],
                                 start=(pm == 0), stop=(pm == 15))
                nc.tensor.matmul(pah1, v2b[pm], pP[:, 512:1024],
                                 start=(pm == 0), stop=(pm == 15))
                if pm == 15:
                    # normalize attnT / den (den = row 64); overlaps the
                    # next n-chunk's score/exp stream on DVE+Pool.
                    nrm = npool.tile([128, 512], F32R, tag="nrm",
                                     name=f"{r}nrm{pn}")
                    for hi, a in ((0, pah0), (1, pah1)):
                        rd = spool.tile([1, 512], F32, tag=f"rd{hi}",
                                        name=f"{r}rd{hi}_{pn}")
                        nc.vector.reciprocal(rd, a[64:65, :])
                        rb = spool.tile([64, 512], F32, tag=f"rb{hi}",
                                        name=f"{r}rb{hi}_{pn}")
                        nc.gpsimd.partition_broadcast(rb, rd)
                        nc.vector.tensor_tensor(nrm[hi * 64:(hi + 1) * 64, :],
                                                a[0:64, :], rb, op=MULT)
                    pending_out = (pn, nrm)

            for n in range(4):
                ns = slice(n * 512, (n + 1) * 512)
                ah0 = psB.tile([65, 512], F32, tag="acc", name=f"{r}ah0_{n}")
                ah1 = psB.tile([65, 512], F32, tag="acc", name=f"{r}ah1_{n}")
                for m in range(16):
                    ms = slice(m * 128, (m + 1) * 128)
                    blk = psA.tile([128, 1024], F32, tag="blk",
                                   name=f"{r}blk{n}_{m}")
                    nc.tensor.matmul(blk[:, 0:512], kT[0:64, ms], qT[0:64, ns],
                                     start=True, stop=True, tile_position=(0, 0))
                    nc.tensor.matmul(blk[:, 512:1024], kT[64:128, ms],
                                     qT[64:128, ns],
                                     start=True, stop=True, tile_position=(64, 0))
                    P = ppool.tile([128, 1024], F32R, tag="p", name=f"{r}p{n}_{m}")
                    nc.scalar.activation(P, blk, EXP)
                    if m >= 1 and fills[n]:
                        for task in fills[n].pop(0):
                            task()
                    if prev is not None:
                        emit_attnout(*prev)
                    prev = (n, m, P, ah0, ah1)
                    if pending_out is not None and m in (3, 6, 9, 12):
                        po_n, po_nrm = pending_out
                        emit_outproj(po_n, po_nrm, (m - 3) // 3)
            # drain the lagged block, then the last n-chunk's outproj
            emit_attnout(*prev)
            po_n, po_nrm = pending_out
            for cc in range(4):
                emit_outproj(po_n, po_nrm, cc)

    nc.compile()
    return nc


def _get_program():
    global _NC
    if _NC is None:
        _NC = _build_program()
    return _NC


def make_in_maps(inputs):
    x = np.asarray(inputs["x"], np.float32)
    y = np.asarray(inputs["y"], np.float32)
    Wq = np.asarray(inputs["Wq"], np.float32)
    Wkv = np.asarray(inputs["Wkv"], np.float32)
    lw = np.asarray(inputs["lw"], np.float32)

    d = np.arange(HD)
    ones = np.ones((128, 1), np.float32)
    xr = [to_fp32r(x[b]) for b in range(B)]
    yr = [to_fp32r(y[b]) for b in range(B)]
    in_maps = []
    for core in range(NCORES):
        b = core // 4
        h0 = (core % 4) * 2
        ch = np.concatenate([h * HD + d for h in (h0, h0 + 1)])  # channels
        colsK = np.concatenate([h * 2 * HD + 2 * d for h in (h0, h0 + 1)])
        wq_c = Wq[:, ch] * np.float32(SCALE)
        wk_c = Wkv[:, colsK]
        wv_c = Wkv[:, colsK + 1] * (1.0 + lw[ch])[None, :]
        wp_c = np.asarray(inputs["Wp"], np.float32)[ch, :]
        in_maps.append({
            "xr": xr[b],
            "yr": yr[b],
            "wq": to_fp32r(wq_c),
            "wk": to_fp32r(wk_c),
            "wv": to_fp32r(wv_c),
            "wp": to_fp32r(wp_c),
            "ones_d": ones,
        })
    return in_maps


def assemble_output(results, inputs):
    lb = np.asarray(inputs["lb"], np.float32)
    Wp = np.asarray(inputs["Wp"], np.float32)
    bp = np.asarray(inputs["bp"], np.float32)
    bias = (bp + lb @ Wp).astype(np.float32)
    out = np.stack([
        results[0]["outT"] + results[1]["outT"]
        + results[2]["outT"] + results[3]["outT"],
        results[4]["outT"] + results[5]["outT"]
        + results[6]["outT"] + results[7]["outT"],
    ])
    out += bias[None, :, None]
    return out.astype(np.float32)


def kernel(x, y, Wq, Wkv, lw, lb, Wp, bp):
    global LAST_RUN
    from concourse.bass_utils import run_bass_kernel_spmd

    inputs = dict(x=x, y=y, Wq=Wq, Wkv=Wkv, lw=lw, lb=lb, Wp=Wp, bp=bp)
    nc = _get_program()
    in_maps = make_in_maps(inputs)
    LAST_RUN = run_bass_kernel_spmd(nc, in_maps, list(range(NCORES)))
    return assemble_output(LAST_RUN.results, inputs)



# revision 10
# speedup vs baseline: 1.0150x; 1.0150x over previous
"""Trainium2 Bass kernel for nn_CrossAttention (B=2, C=512, N=M=2048, H=8).

Sharding: batch*heads = 16 (b,h) pairs across 8 cores, 2 heads per core.
Cores 0-3 handle batch 0 (heads 0..7 in pairs), cores 4-7 batch 1.

v2 design (ScalarE-exp-bound; PE work cut under the exp floor):
  qT[d,n] = (Wq*SCALE).T @ x_b   (f32r weights, bf16 x moving)    8192c
  kT[d,m] = Wk.T @ y_b                                            8192c
  v2[m,d] = y_b.T-slices @ (Wv*(1+lw))   direct [m,d] layout,     8320c
            bf16, no PE transposes; ones cols for the denominator
  S^T[m,n] = kT.T-slices @ qT   (K=64 pairs tile_position-packed) 65536c
  P = exp(S^T) -> bf16          (ScalarE, 64x [128,1024] blocks — the
                                 hard floor: 65536 rows @ 0.833ns)
  att[n, d|den] += P_slice.T @ v2[m]   n-major: out 128 partitions,
            65-row matmuls, bf16                                  33280c
  att_nrm[n,d2] = att * recip(den)     (DVE recip + Pool t-scalar)
  attT[d2,n] = transpose(att_nrm)      (PE, bf16 identity)         2048c
  outT_partial[c,n] = Wp_rows.T @ attT  (bf16)                     8192c

The depthwise conv (ksize=1) folds into Wv scaling + a host-side output
bias (bias' = bp + lb @ Wp, exact because softmax rows sum to 1).
Host sums the 4 per-batch partials and adds bias'.

PSUM: psA 3x[128,1024] ring (scores/exp; also proj, v2, transposes and
outproj transients) = 6 banks; psB 2x[128,512] = 2 banks holding the
8 per-chunk attnout accumulators (4x65 cols per bank, pending-zero
start-once trick for co-located accumulation groups).
"""

import os
import sys
import numpy as np
from contextlib import ExitStack

for _p in ("/root/.axon_site", "/root/.axon_site/_ro/trn_rl_repo",
           "/root/.axon_site/_ro/pypackages", "/opt/trn_rl_repo"):
    if os.path.isdir(_p) and _p not in sys.path:
        sys.path.append(_p)

B, C, N, M, H = 2, 512, 2048, 2048, 8
HD = C // H
SCALE = HD ** -0.5
NCORES = 8

_NC = None
LAST_RUN = None


def to_fp32r(x: np.ndarray) -> np.ndarray:
    """Round fp32 to the 20-bit (1s/8e/11m) fp32r grid, round-to-nearest-even."""
    b = np.ascontiguousarray(x, np.float32).view(np.uint32).astype(np.uint64)
    rb = (b >> 12) & 1
    b = (b + 0x7FF + rb) & 0xFFFFF000
    return b.astype(np.uint32).view(np.float32)


def _build_program():
    from concourse import bacc
    import concourse.tile as tile
    import concourse.mybir as mybir
    from concourse.masks import make_identity

    F32 = mybir.dt.float32
    F32R = mybir.dt.float32r
    BF16 = mybir.dt.bfloat16
    EXP = mybir.ActivationFunctionType.Exp
    MULT = mybir.AluOpType.mult

    nc = bacc.Bacc("TRN2", target_bir_lowering=False, debug=False,
                   num_devices=NCORES)

    xr = nc.dram_tensor("xr", [C, N], BF16, kind="ExternalInput").ap()
    yr = nc.dram_tensor("yr", [C, M], BF16, kind="ExternalInput").ap()
    wq_d = nc.dram_tensor("wq", [C, 128], BF16, kind="ExternalInput").ap()
    wk_d = nc.dram_tensor("wk", [C, 128], BF16, kind="ExternalInput").ap()
    wv_d = nc.dram_tensor("wv", [C, 128], BF16, kind="ExternalInput").ap()
    wp_d = nc.dram_tensor("wp", [128, C], BF16, kind="ExternalInput").ap()
    outT = nc.dram_tensor("outT", [C, N], F32, kind="ExternalOutput").ap()

    with tile.TileContext(nc) as tc, ExitStack() as ctx:
        sb = ctx.enter_context(tc.tile_pool(name="sb", bufs=1))
        ppool = ctx.enter_context(tc.tile_pool(name="ppool", bufs=3))
        npool = ctx.enter_context(tc.tile_pool(name="npool", bufs=4))
        apool = ctx.enter_context(tc.tile_pool(name="apool", bufs=4))
        spool = ctx.enter_context(tc.tile_pool(name="spool", bufs=4))
        opool = ctx.enter_context(tc.tile_pool(name="opool", bufs=3))
        psA = ctx.enter_context(tc.tile_pool(name="psA", bufs=3, space="PSUM"))
        psB = ctx.enter_context(tc.tile_pool(name="psB", bufs=2, space="PSUM"))

        # ---- constants ----
        ident = sb.tile([128, 128], BF16, tag="ident")
        make_identity(nc, ident)
        # v2 tiles: [m 128, 130] bf16; cols 64/129 stay 1.0 (denominator)
        v2 = [sb.tile([128, 130], BF16, tag=f"v2_{m}", name=f"v2_{m}")
              for m in range(16)]
        for m in range(16):
            nc.gpsimd.memset(v2[m], 1.0)
        # warm the exp table while DMAs stream
        warm = sb.tile([1, 32], F32, tag="warm")
        nc.scalar.activation(warm, ident[0:1, 0:32], EXP)
        # warm the PE clock so early projections run fast
        psw = psB.tile([128, 512], F32, tag="acc", name="psw")
        for _ in range(8):
            nc.tensor.matmul(psw[:, 0:128], ident, ident, start=True, stop=True)
        warm2 = sb.tile([128, 128], F32, tag="warm2")
        nc.vector.tensor_copy(warm2, psw[:, 0:128])

        wq_sb = sb.tile([128, 4, 128], BF16, tag="wq_sb")
        wk_sb = sb.tile([128, 4, 128], BF16, tag="wk_sb")
        wv_sb = sb.tile([128, 4, 128], BF16, tag="wv_sb")
        wp_sb = sb.tile([128, C], BF16, tag="wp_sb")

        y_sb = [sb.tile([128, M], BF16, tag=f"y_sb{k}", name=f"y_sb{k}")
                for k in range(4)]
        x_sb = [sb.tile([128, N], BF16, tag=f"x_sb{k}", name=f"x_sb{k}")
                for k in range(4)]

        def load_slices(dst_tiles, src, j):
            js = slice(j * 512, (j + 1) * 512)
            for k in range(4):
                nc.sync.dma_start(out=dst_tiles[k][:, js],
                                  in_=src[k * 128:(k + 1) * 128, js])

        # DMA order = consumption order
        nc.sync.dma_start(
            out=wk_sb, in_=wk_d.rearrange("(kc p) m -> p kc m", p=128))
        load_slices(y_sb, yr, 0)
        nc.sync.dma_start(
            out=wv_sb, in_=wv_d.rearrange("(kc p) m -> p kc m", p=128))
        nc.sync.dma_start(
            out=wq_sb, in_=wq_d.rearrange("(kc p) m -> p kc m", p=128))
        load_slices(x_sb, xr, 0)
        load_slices(y_sb, yr, 1)
        load_slices(y_sb, yr, 2)
        load_slices(y_sb, yr, 3)
        nc.sync.dma_start(out=wp_sb, in_=wp_d)
        load_slices(x_sb, xr, 1)
        load_slices(x_sb, xr, 2)
        load_slices(x_sb, xr, 3)

        kT = sb.tile([128, M], F32R, tag="kT")
        qT = sb.tile([128, N], F32R, tag="qT")

        def proj(dst, w_sb, src, j, name):
            ps = psA.tile([128, 512], F32, tag="blk", name=name)
            for kc in range(4):
                nc.tensor.matmul(ps, w_sb[:, kc, :],
                                 src[kc][:, j * 512:(j + 1) * 512],
                                 start=(kc == 0), stop=(kc == 3))
            nc.vector.tensor_copy(dst[:, j * 512:(j + 1) * 512], ps)

        def v2_proj(m):
            ps = psA.tile([128, 128], F32, tag="blk", name=f"psv{m}")
            for kc in range(4):
                nc.tensor.matmul(ps, y_sb[kc][:, m * 128:(m + 1) * 128],
                                 wv_sb[:, kc, :],
                                 start=(kc == 0), stop=(kc == 3))
            nc.vector.tensor_copy(v2[m][:, 0:64], ps[:, 0:64])
            nc.vector.tensor_copy(v2[m][:, 65:129], ps[:, 64:128])

        # ---- prologue: j0 projections + first v2 blocks ----
        proj(kT, wk_sb, y_sb, 0, "psk0")
        for m in range(4):
            v2_proj(m)
        proj(qT, wq_sb, x_sb, 0, "psq0")

        # fill task groups woven between score blocks (chunk -> per-m lists)
        fills = {
            0: [[lambda: proj(kT, wk_sb, y_sb, 1, "psk1")],
                [lambda: v2_proj(4)],
                [lambda: v2_proj(5)],
                [lambda: v2_proj(6)],
                [lambda: v2_proj(7)],
                [lambda: proj(kT, wk_sb, y_sb, 2, "psk2")],
                [lambda: v2_proj(8)],
                [lambda: v2_proj(9)],
                [lambda: v2_proj(10)],
                [lambda: v2_proj(11)],
                [lambda: proj(kT, wk_sb, y_sb, 3, "psk3")],
                [lambda: v2_proj(12)],
                [lambda: v2_proj(13)],
                [lambda: v2_proj(14)],
                [lambda: proj(qT, wq_sb, x_sb, 1, "psq1"),
                 lambda: v2_proj(15)]],
            1: [[lambda: proj(qT, wq_sb, x_sb, 2, "psq2")]],
            2: [[lambda: proj(qT, wq_sb, x_sb, 3, "psq3")]],
            3: [],
        }

        # ---- attention main loop ----
        # attnout lags one m-step behind scores/exp; the drain of chunk q
        # (normalize/transpose/outproj/DMA) is woven into chunk q+1's first
        # m-steps so the PE's in-order queue never stalls the score stream.
        prev = None          # (m, P, accA, accB) awaiting attnout
        drain = None         # chunk state awaiting normalize/outproj

        def emit_attnout(pm, pP, paccA, paccB):
            # pm==0 is the first matmul into each fresh acc bank: its
            # start=True marks the whole bank pending-zero; later groups'
            # first writes then init via the per-byte pending-zero path.
            for nb in range(4):
                for h, acc in ((0, paccA), (1, paccB)):
                    nc.tensor.matmul(
                        acc[:, nb * 65:(nb + 1) * 65],
                        pP[:, h * 512 + nb * 128: h * 512 + (nb + 1) * 128],
                        v2[pm][:, h * 65: h * 65 + 65],
                        start=(pm == 0 and nb == 0),
                        stop=(pm == 15 and nb == 3 and h == 1),
                        skip_group_check=True)

        def emit_norm(q, qaccA, qaccB):
            # recip + per-partition-scalar multiply; att_nrm [n,128] bf16
            nrms = []
            for nb in range(4):
                nrm = npool.tile([128, 128], BF16, tag="nrm",
                                 name=f"nrm{q}_{nb}")
                for h, acc in ((0, qaccA), (1, qaccB)):
                    rd = spool.tile([128, 1], F32, tag="rd",
                                    name=f"rd{q}_{nb}_{h}")
                    nc.vector.reciprocal(rd, acc[:, nb * 65 + 64: nb * 65 + 65])
                    nc.vector.tensor_scalar(
                        nrm[:, h * 64:(h + 1) * 64],
                        acc[:, nb * 65: nb * 65 + 64], rd, None, op0=MULT)
                nrms.append(nrm)
            return nrms

        def emit_transposes(q, nrms):
            attTs = []
            for nb in range(4):
                tp = psA.tile([128, 128], BF16, tag="blk", name=f"tp{q}_{nb}")
                nc.tensor.transpose(tp, nrms[nb], ident)
                at = apool.tile([128, 128], BF16, tag="attT",
                                name=f"attT{q}_{nb}")
                nc.vector.tensor_copy(at, tp)
                attTs.append(at)
            return attTs

        def emit_outproj(q, attTs, cb):
            po = psA.tile([128, 512], F32, tag="blk", name=f"po{q}_{cb}")
            for nb in range(4):
                nc.tensor.matmul(po[:, nb * 128:(nb + 1) * 128],
                                 wp_sb[:, cb * 128:(cb + 1) * 128],
                                 attTs[nb],
                                 start=(nb == 0), stop=(nb == 3),
                                 skip_group_check=True)
            so = opool.tile([128, 512], F32, tag="so", name=f"so{q}_{cb}")
            nc.vector.tensor_copy(so, po)
            nc.sync.dma_start(
                out=outT[cb * 128:(cb + 1) * 128, q * 512:(q + 1) * 512],
                in_=so)

        for n in range(4):
            ns = slice(n * 512, (n + 1) * 512)
            accA = psB.tile([128, 512], F32, tag="acc", name=f"accA{n}")
            accB = psB.tile([128, 512], F32, tag="acc", name=f"accB{n}")
            for m in range(16):
                ms = slice(m * 128, (m + 1) * 128)
                blk = psA.tile([128, 1024], F32, tag="blk",
                               name=f"blk{n}_{m}")
                nc.tensor.matmul(blk[:, 0:512], kT[0:64, ms], qT[0:64, ns],
                                 start=True, stop=True, tile_position=(0, 0))
                nc.tensor.matmul(blk[:, 512:1024], kT[64:128, ms],
                                 qT[64:128, ns],
                                 start=True, stop=True, tile_position=(64, 0))
                P = ppool.tile([128, 1024], BF16, tag="p", name=f"p{n}_{m}")
                nc.scalar.activation(P, blk, EXP)
                if prev is not None:
                    emit_attnout(*prev)
                    if prev[0] == 15:
                        # chunk n-1 fully accumulated: kick its normalize
                        drain = (n - 1, emit_norm(n - 1, prev[2], prev[3]))
                prev = (m, P, accA, accB)
                if m >= 1 and fills[n]:
                    for task in fills[n].pop(0):
                        task()
                if drain is not None:
                    dq, dstate = drain
                    if m == 1:
                        drain = (dq, emit_transposes(dq, dstate))
                    elif 2 <= m <= 5:
                        emit_outproj(dq, dstate, m - 2)
                        if m == 5:
                            drain = None
            # (chunk-end work is deferred into the next chunk's m-steps)

        # ---- epilogue: drain the final chunk ----
        emit_attnout(*prev)
        nrms = emit_norm(3, prev[2], prev[3])
        attTs = emit_transposes(3, nrms)
        for cb in range(4):
            emit_outproj(3, attTs, cb)

    nc.compile()
    return nc


def _get_program():
    global _NC
    if _NC is None:
        _NC = _build_program()
    return _NC


def make_in_maps(inputs):
    import ml_dtypes
    bf16 = ml_dtypes.bfloat16

    x = np.asarray(inputs["x"], np.float32)
    y = np.asarray(inputs["y"], np.float32)
    Wq = np.asarray(inputs["Wq"], np.float32)
    Wkv = np.asarray(inputs["Wkv"], np.float32)
    lw = np.asarray(inputs["lw"], np.float32)
    Wp = np.asarray(inputs["Wp"], np.float32)

    d = np.arange(HD)
    xr = [np.ascontiguousarray(x[b].astype(bf16)) for b in range(B)]
    yr = [np.ascontiguousarray(y[b].astype(bf16)) for b in range(B)]
    in_maps = []
    for core in range(NCORES):
        b = core // 4
        h0 = (core % 4) * 2
        ch = np.concatenate([h * HD + d for h in (h0, h0 + 1)])  # channels
        colsK = np.concatenate([h * 2 * HD + 2 * d for h in (h0, h0 + 1)])
        wq_c = Wq[:, ch] * np.float32(SCALE)
        wk_c = Wkv[:, colsK]
        wv_c = Wkv[:, colsK + 1] * (1.0 + lw[ch])[None, :]
        in_maps.append({
            "xr": xr[b],
            "yr": yr[b],
            "wq": np.ascontiguousarray(wq_c.astype(bf16)),
            "wk": np.ascontiguousarray(wk_c.astype(bf16)),
            "wv": np.ascontiguousarray(wv_c.astype(bf16)),
            "wp": np.ascontiguousarray(Wp[ch, :].astype(bf16)),
        })
    return in_maps


def assemble_output(results, inputs):
    lb = np.asarray(inputs["lb"], np.float32)
    Wp = np.asarray(inputs["Wp"], np.float32)
    bp = np.asarray(inputs["bp"], np.float32)
    bias = (bp + lb @ Wp).astype(np.float32)
    out = np.stack([
        results[0]["outT"] + results[1]["outT"]
        + results[2]["outT"] + results[3]["outT"],
        results[4]["outT"] + results[5]["outT"]
        + results[6]["outT"] + results[7]["outT"],
    ])
    out += bias[None, :, None]
    return out.astype(np.float32)


def kernel(x, y, Wq, Wkv, lw, lb, Wp, bp):
    global LAST_RUN
    from concourse.bass_utils import run_bass_kernel_spmd

    inputs = dict(x=x, y=y, Wq=Wq, Wkv=Wkv, lw=lw, lb=lb, Wp=Wp, bp=bp)
    nc = _get_program()
    in_maps = make_in_maps(inputs)
    LAST_RUN = run_bass_kernel_spmd(nc, in_maps, list(range(NCORES)))
    return assemble_output(LAST_RUN.results, inputs)


# revision 12
# speedup vs baseline: 1.0691x; 1.0534x over previous
"""Trainium2 Bass kernel for nn_CrossAttention (B=2, C=512, N=M=2048, H=8).

Sharding: batch*heads = 16 (b,h) pairs across 8 cores, 2 heads per core.
Cores 0-3 handle batch 0 (heads 0..7 in pairs), cores 4-7 batch 1.

The kernel is ScalarE-exp-bound (softmax needs 65536 exp rows/core at
0.833ns — a ~55us engine floor no other engine can take), so PE work is
restructured to fit under it:
  qT[d,n] = (Wq*SCALE).T @ x_b   (bf16)                           8192c
  kT[d,m] = Wk.T @ y_b           (bf16, f32r in SBUF)             8192c
  v2[m,d] = y_b.T-slices @ (Wv*(1+lw))  direct [m,d] layout,      8192c
            bf16, no PE transposes; ones cols give the denominator
  S^T[m,n] = kT.T-slices @ qT   (K=64 pairs tile_position-packed) 65536c
  P = exp(S^T) -> bf16          (ScalarE, 64x [128,1024] blocks)
  att[n, d|den] += P_slice.T @ v2[m]   n-major: 128 out partitions,
            65-row bf16 matmuls (half the m-major cost)           33280c
  att_nrm[n,d2] = att * recip(den)     (DVE, per-partition scalar)
  attT[d2,n] = transpose(att_nrm)      (PE, bf16 identity)         2048c
  outT_partial[c,n] = Wp_rows.T @ attT  (bf16) -> f16 partials     8192c

The depthwise conv (ksize=1) folds into Wv scaling + a host-side output
bias (bias' = bp + lb @ Wp, exact because softmax rows sum to 1).
Host sums the 4 per-batch f16 partials in f32 and adds bias'.

PSUM: psA 3x[128,1024] ring (scores/exp; also proj, v2, transposes and
outproj transients) = 6 banks; psB 2x[128,512] = 2 banks holding the
8 per-chunk attnout accumulators (4x65 cols per bank; only the first
matmul into a bank uses start=True — the bank-wide pending-zero then
zero-initializes each co-located accumulation group on first touch).

Chunk q's drain (normalize/transpose/outproj/DMA) is woven into chunk
q+1's first m-steps so the PE's in-order queue and the psA ring never
stall the score stream that feeds ScalarE.
"""

import os
import sys
import numpy as np
from contextlib import ExitStack

for _p in ("/root/.axon_site", "/root/.axon_site/_ro/trn_rl_repo",
           "/root/.axon_site/_ro/pypackages", "/opt/trn_rl_repo"):
    if os.path.isdir(_p) and _p not in sys.path:
        sys.path.append(_p)

B, C, N, M, H = 2, 512, 2048, 2048, 8
HD = C // H
SCALE = HD ** -0.5
NCORES = 8

_NC = None
LAST_RUN = None


def _build_program():
    from concourse import bacc
    import concourse.tile as tile
    import concourse.mybir as mybir
    from concourse.masks import make_identity

    F32 = mybir.dt.float32
    F32R = mybir.dt.float32r
    BF16 = mybir.dt.bfloat16
    F16 = mybir.dt.float16
    EXP = mybir.ActivationFunctionType.Exp
    MULT = mybir.AluOpType.mult

    nc = bacc.Bacc("TRN2", target_bir_lowering=False, debug=False,
                   num_devices=NCORES)

    xr = nc.dram_tensor("xr", [C, N], BF16, kind="ExternalInput").ap()
    yr = nc.dram_tensor("yr", [C, M], BF16, kind="ExternalInput").ap()
    wq_d = nc.dram_tensor("wq", [C, 128], BF16, kind="ExternalInput").ap()
    wk_d = nc.dram_tensor("wk", [C, 128], BF16, kind="ExternalInput").ap()
    wv_d = nc.dram_tensor("wv", [C, 128], BF16, kind="ExternalInput").ap()
    wp_d = nc.dram_tensor("wp", [128, C], BF16, kind="ExternalInput").ap()
    outT = nc.dram_tensor("outT", [C, N], F16, kind="ExternalOutput").ap()

    with tile.TileContext(nc) as tc, ExitStack() as ctx:
        sb = ctx.enter_context(tc.tile_pool(name="sb", bufs=1))
        ppool = ctx.enter_context(tc.tile_pool(name="ppool", bufs=3))
        npool = ctx.enter_context(tc.tile_pool(name="npool", bufs=4))
        apool = ctx.enter_context(tc.tile_pool(name="apool", bufs=2))
        spool = ctx.enter_context(tc.tile_pool(name="spool", bufs=2))
        opool = ctx.enter_context(tc.tile_pool(name="opool", bufs=2))
        psA = ctx.enter_context(tc.tile_pool(name="psA", bufs=3, space="PSUM"))
        psB = ctx.enter_context(tc.tile_pool(name="psB", bufs=2, space="PSUM"))

        # ---- constants ----
        ident = sb.tile([128, 128], BF16, tag="ident")
        make_identity(nc, ident)
        # v2 tiles: [m 128, 130] bf16; cols 64/129 stay 1.0 (denominator)
        v2 = [sb.tile([128, 130], BF16, tag=f"v2_{m}", name=f"v2_{m}")
              for m in range(16)]
        for m in range(16):
            nc.gpsimd.memset(v2[m], 1.0)
        # warm the exp table while DMAs stream
        warm = sb.tile([1, 32], F32, tag="warm")
        nc.scalar.activation(warm, ident[0:1, 0:32], EXP)
        # warm the PE clock so early projections run fast
        psw = psB.tile([128, 512], F32, tag="acc", name="psw")
        for _ in range(8):
            nc.tensor.matmul(psw[:, 0:128], ident, ident, start=True, stop=True)
        warm2 = sb.tile([128, 128], F32, tag="warm2")
        nc.vector.tensor_copy(warm2, psw[:, 0:128])

        wq_sb = sb.tile([128, 4, 128], BF16, tag="wq_sb")
        wk_sb = sb.tile([128, 4, 128], BF16, tag="wk_sb")
        wv_sb = sb.tile([128, 4, 128], BF16, tag="wv_sb")
        wp_sb = sb.tile([128, C], BF16, tag="wp_sb")

        y_sb = sb.tile([128, 4, M], BF16, tag="y_sb")
        x_sb = sb.tile([128, 4, N], BF16, tag="x_sb")

        def load_j(dst, src, j):
            js = slice(j * 512, (j + 1) * 512)
            nc.sync.dma_start(
                out=dst[:, :, js],
                in_=src[:, js].rearrange("(kc p) m -> p kc m", p=128))

        # DMA order = consumption order; one DMA per j-chunk
        nc.sync.dma_start(
            out=wk_sb, in_=wk_d.rearrange("(kc p) m -> p kc m", p=128))
        load_j(y_sb, yr, 0)
        nc.sync.dma_start(
            out=wv_sb, in_=wv_d.rearrange("(kc p) m -> p kc m", p=128))
        nc.sync.dma_start(
            out=wq_sb, in_=wq_d.rearrange("(kc p) m -> p kc m", p=128))
        load_j(x_sb, xr, 0)
        load_j(y_sb, yr, 1)
        load_j(y_sb, yr, 2)
        load_j(y_sb, yr, 3)
        nc.sync.dma_start(out=wp_sb, in_=wp_d)
        load_j(x_sb, xr, 1)
        load_j(x_sb, xr, 2)
        load_j(x_sb, xr, 3)

        kT = sb.tile([128, M], F32R, tag="kT")
        qT = sb.tile([128, N], F32R, tag="qT")

        def proj(dst, w_sb, src, j, name):
            ps = psA.tile([128, 512], F32, tag="blk", name=name)
            for kc in range(4):
                nc.tensor.matmul(ps, w_sb[:, kc, :],
                                 src[:, kc, j * 512:(j + 1) * 512],
                                 start=(kc == 0), stop=(kc == 3))
            nc.vector.tensor_copy(dst[:, j * 512:(j + 1) * 512], ps)

        def v2_proj(m):
            ps = psA.tile([128, 128], F32, tag="blk", name=f"psv{m}")
            for kc in range(4):
                nc.tensor.matmul(ps, y_sb[:, kc, m * 128:(m + 1) * 128],
                                 wv_sb[:, kc, :],
                                 start=(kc == 0), stop=(kc == 3))
            nc.vector.tensor_copy(v2[m][:, 0:64], ps[:, 0:64])
            nc.vector.tensor_copy(v2[m][:, 65:129], ps[:, 64:128])

        # ---- prologue: j0 projections + first v2 blocks ----
        proj(kT, wk_sb, y_sb, 0, "psk0")
        for m in range(4):
            v2_proj(m)
        proj(qT, wq_sb, x_sb, 0, "psq0")

        # fill task groups woven between score blocks (chunk -> per-m lists)
        fills = {
            0: [[lambda: proj(kT, wk_sb, y_sb, 1, "psk1")],
                [lambda: v2_proj(4)],
                [lambda: v2_proj(5)],
                [lambda: v2_proj(6)],
                [lambda: v2_proj(7)],
                [lambda: proj(kT, wk_sb, y_sb, 2, "psk2")],
                [lambda: v2_proj(8)],
                [lambda: v2_proj(9)],
                [lambda: v2_proj(10)],
                [lambda: v2_proj(11)],
                [lambda: proj(kT, wk_sb, y_sb, 3, "psk3")],
                [lambda: v2_proj(12)],
                [lambda: v2_proj(13)],
                [lambda: v2_proj(14)],
                [lambda: proj(qT, wq_sb, x_sb, 1, "psq1"),
                 lambda: v2_proj(15)]],
            1: [[lambda: proj(qT, wq_sb, x_sb, 2, "psq2")]],
            2: [[lambda: proj(qT, wq_sb, x_sb, 3, "psq3")]],
            3: [],
        }

        # ---- attention main loop ----
        prev = None          # (m, P, accA, accB) awaiting attnout
        drain = None         # [stage, chunk, state...] of the pending drain

        def emit_attnout(pm, pP, paccA, paccB):
            # pm==0/nb==0 is the first matmul into each fresh acc bank: its
            # start=True marks the whole bank pending-zero; later groups'
            # first writes then zero-init via the per-byte pending path.
            for nb in range(4):
                for h, acc in ((0, paccA), (1, paccB)):
                    nc.tensor.matmul(
                        acc[:, nb * 65:(nb + 1) * 65],
                        pP[:, h * 512 + nb * 128: h * 512 + (nb + 1) * 128],
                        v2[pm][:, h * 65: h * 65 + 65],
                        start=(pm == 0 and nb == 0),
                        stop=(pm == 15 and nb == 3 and h == 1),
                        skip_group_check=True)

        def emit_norm(q, qaccA, qaccB, use_act=False):
            # batched strided reciprocal of the 4 denominator columns per
            # bank, then 8 per-partition-scalar multiplies -> bf16
            nrm = npool.tile([128, 512], BF16, tag="nrm", name=f"nrm{q}")
            rds = []
            for h, acc in ((0, qaccA), (1, qaccB)):
                rd = spool.tile([128, 4], F32, tag=f"rd{h}", name=f"rd{q}_{h}")
                nc.vector.reciprocal(rd, acc[:, 64:261:65])
                rds.append(rd)
            for nb in range(4):
                for h, acc in ((0, qaccA), (1, qaccB)):
                    dst = nrm[:, nb * 128 + h * 64: nb * 128 + (h + 1) * 64]
                    src = acc[:, nb * 65: nb * 65 + 64]
                    if use_act and h == 1:
                        nc.scalar.mul(dst, src, rds[h][:, nb:nb + 1])
                    else:
                        nc.vector.tensor_scalar(dst, src,
                                                rds[h][:, nb:nb + 1], None,
                                                op0=MULT)
            return nrm

        def emit_transposes(q, nrm):
            # 4 transposes share one PSUM slot; one bf16 2x copy out
            tp = psA.tile([128, 512], BF16, tag="blk", name=f"tp{q}")
            for nb in range(4):
                nc.tensor.transpose(
                    tp[:, nb * 128:(nb + 1) * 128],
                    nrm[:, nb * 128:(nb + 1) * 128], ident)
            at = apool.tile([128, 512], BF16, tag="attT", name=f"attT{q}")
            nc.vector.tensor_copy(at, tp)
            return at

        def emit_outproj(q, at, half, so, use_act=False):
            # two output-channel blocks share one PSUM slot -> f16 halves
            po = psA.tile([128, 1024], F32, tag="blk", name=f"po{q}_{half}")
            for i in range(2):
                cb = half * 2 + i
                for nb in range(4):
                    nc.tensor.matmul(
                        po[:, i * 512 + nb * 128: i * 512 + (nb + 1) * 128],
                        wp_sb[:, cb * 128:(cb + 1) * 128],
                        at[:, nb * 128:(nb + 1) * 128],
                        start=(nb == 0), stop=(nb == 3 and i == 1),
                        skip_group_check=True)
            if use_act:
                nc.scalar.copy(so[:, half * 1024:(half + 1) * 1024], po)
            else:
                nc.vector.tensor_copy(so[:, half * 1024:(half + 1) * 1024], po)

        def emit_outdma(q, so):
            nc.sync.dma_start(
                out=outT[:, q * 512:(q + 1) * 512].rearrange(
                    "(cb p) n -> p cb n", p=128),
                in_=so.rearrange("p (cb n) -> p cb n", n=512))

        for n in range(4):
            ns = slice(n * 512, (n + 1) * 512)
            accA = psB.tile([128, 512], F32, tag="acc", name=f"accA{n}")
            accB = psB.tile([128, 512], F32, tag="acc", name=f"accB{n}")
            for m in range(16):
                ms = slice(m * 128, (m + 1) * 128)
                blk = psA.tile([128, 1024], F32, tag="blk",
                               name=f"blk{n}_{m}")
                nc.tensor.matmul(blk[:, 0:512], kT[0:64, ms], qT[0:64, ns],
                                 start=True, stop=True, tile_position=(0, 0))
                nc.tensor.matmul(blk[:, 512:1024], kT[64:128, ms],
                                 qT[64:128, ns],
                                 start=True, stop=True, tile_position=(64, 0))
                P = ppool.tile([128, 1024], BF16, tag="p", name=f"p{n}_{m}")
                nc.scalar.activation(P, blk, EXP)
                if prev is not None:
                    emit_attnout(*prev)
                    if prev[0] == 15:
                        # chunk n-1 fully accumulated: kick its normalize
                        drain = [0, n - 1,
                                 emit_norm(n - 1, prev[2], prev[3]), None]
                prev = (m, P, accA, accB)
                if m >= 1 and fills[n]:
                    for task in fills[n].pop(0):
                        task()
                if drain is not None:
                    stage, dq, dstate, dso = drain
                    if stage == 0 and m >= 1:
                        drain = [1, dq, emit_transposes(dq, dstate),
                                 opool.tile([128, 2048], F16, tag="so",
                                            name=f"so{dq}")]
                    elif stage == 1:
                        emit_outproj(dq, dstate, 0, dso)
                        drain[0] = 2
                    elif stage == 2:
                        emit_outproj(dq, dstate, 1, dso)
                        emit_outdma(dq, dso)
                        drain = None

        # ---- epilogue: drain the final chunk (ScalarE is idle now, so it
        # takes half the normalize and one outproj copy) ----
        emit_attnout(*prev)
        nrm = emit_norm(3, prev[2], prev[3], use_act=True)
        at = emit_transposes(3, nrm)
        so = opool.tile([128, 2048], F16, tag="so", name="so3")
        emit_outproj(3, at, 0, so, use_act=True)
        emit_outproj(3, at, 1, so)
        emit_outdma(3, so)

    nc.compile()
    return nc


def _get_program():
    global _NC
    if _NC is None:
        _NC = _build_program()
    return _NC


def make_in_maps(inputs):
    import ml_dtypes
    bf16 = ml_dtypes.bfloat16

    x = np.asarray(inputs["x"], np.float32)
    y = np.asarray(inputs["y"], np.float32)
    Wq = np.asarray(inputs["Wq"], np.float32)
    Wkv = np.asarray(inputs["Wkv"], np.float32)
    lw = np.asarray(inputs["lw"], np.float32)
    Wp = np.asarray(inputs["Wp"], np.float32)

    d = np.arange(HD)
    xr = [np.ascontiguousarray(x[b].astype(bf16)) for b in range(B)]
    yr = [np.ascontiguousarray(y[b].astype(bf16)) for b in range(B)]
    in_maps = []
    for core in range(NCORES):
        b = core // 4
        h0 = (core % 4) * 2
        ch = np.concatenate([h * HD + d for h in (h0, h0 + 1)])  # channels
        colsK = np.concatenate([h * 2 * HD + 2 * d for h in (h0, h0 + 1)])
        wq_c = Wq[:, ch] * np.float32(SCALE)
        wk_c = Wkv[:, colsK]
        wv_c = Wkv[:, colsK + 1] * (1.0 + lw[ch])[None, :]
        in_maps.append({
            "xr": xr[b],
            "yr": yr[b],
            "wq": np.ascontiguousarray(wq_c.astype(bf16)),
            "wk": np.ascontiguousarray(wk_c.astype(bf16)),
            "wv": np.ascontiguousarray(wv_c.astype(bf16)),
            "wp": np.ascontiguousarray(Wp[ch, :].astype(bf16)),
        })
    return in_maps


def assemble_output(results, inputs):
    lb = np.asarray(inputs["lb"], np.float32)
    Wp = np.asarray(inputs["Wp"], np.float32)
    bp = np.asarray(inputs["bp"], np.float32)
    bias = (bp + lb @ Wp).astype(np.float32)
    parts = [np.asarray(results[c]["outT"], dtype=np.float32)
             for c in range(NCORES)]
    out = np.stack([parts[0] + parts[1] + parts[2] + parts[3],
                    parts[4] + parts[5] + parts[6] + parts[7]])
    out += bias[None, :, None]
    return out.astype(np.float32)


def kernel(x, y, Wq, Wkv, lw, lb, Wp, bp):
    global LAST_RUN
    from concourse.bass_utils import run_bass_kernel_spmd

    inputs = dict(x=x, y=y, Wq=Wq, Wkv=Wkv, lw=lw, lb=lb, Wp=Wp, bp=bp)
    nc = _get_program()
    in_maps = make_in_maps(inputs)
    LAST_RUN = run_bass_kernel_spmd(nc, in_maps, list(range(NCORES)))
    return assemble_output(LAST_RUN.results, inputs)


# revision 13
# speedup vs baseline: 1.1070x; 1.0355x over previous
"""Trainium2 Bass kernel for nn_CrossAttention (B=2, C=512, N=M=2048, H=8).

Sharding: batch*heads = 16 (b,h) pairs across 8 cores, 2 heads per core.
Cores 0-3 handle batch 0 (heads 0..7 in pairs), cores 4-7 batch 1.

The kernel is ScalarE-exp-bound (softmax needs 65536 exp rows/core at
0.833ns — a ~55us engine floor no other engine can take), so PE work is
restructured to fit under it:
  qT[d,n] = (Wq*SCALE).T @ x_b   (bf16)                           8192c
  kT[d,m] = Wk.T @ y_b           (bf16, f32r in SBUF)             8192c
  v2[m,d] = y_b.T-slices @ (Wv*(1+lw))  direct [m,d] layout,      8192c
            bf16, no PE transposes; ones cols give the denominator
  S^T[m,n] = kT.T-slices @ qT   (K=64 pairs tile_position-packed) 65536c
  P = exp(S^T) -> bf16          (ScalarE, 64x [128,1024] blocks)
  att[n, d|den] += P_slice.T @ v2[m]   n-major: 128 out partitions,
            65-row bf16 matmuls (half the m-major cost)           33280c
  att_nrm[n,d2] = att * recip(den)     (DVE, per-partition scalar)
  attT[d2,n] = transpose(att_nrm)      (PE, bf16 identity)         2048c
  outT_partial[c,n] = Wp_rows.T @ attT  (bf16) -> f16 partials     8192c

The depthwise conv (ksize=1) folds into Wv scaling + a host-side output
bias (bias' = bp + lb @ Wp, exact because softmax rows sum to 1).
Host sums the 4 per-batch f16 partials in f32 and adds bias'.

PSUM: psA 3x[128,1024] ring (scores/exp; also proj, v2, transposes and
outproj transients) = 6 banks; psB 2x[128,512] = 2 banks holding the
8 per-chunk attnout accumulators (4x65 cols per bank; only the first
matmul into a bank uses start=True — the bank-wide pending-zero then
zero-initializes each co-located accumulation group on first touch).

Chunk q's drain (normalize/transpose/outproj/DMA) is woven into chunk
q+1's first m-steps so the PE's in-order queue and the psA ring never
stall the score stream that feeds ScalarE.
"""

import os
import sys
import numpy as np
from contextlib import ExitStack

for _p in ("/root/.axon_site", "/root/.axon_site/_ro/trn_rl_repo",
           "/root/.axon_site/_ro/pypackages", "/opt/trn_rl_repo"):
    if os.path.isdir(_p) and _p not in sys.path:
        sys.path.append(_p)

B, C, N, M, H = 2, 512, 2048, 2048, 8
HD = C // H
SCALE = HD ** -0.5
NCORES = 8

_NC = None
LAST_RUN = None


def _build_program():
    from concourse import bacc
    import concourse.tile as tile
    import concourse.mybir as mybir
    from concourse.masks import make_identity

    F32 = mybir.dt.float32
    F32R = mybir.dt.float32r
    BF16 = mybir.dt.bfloat16
    F16 = mybir.dt.float16
    EXP = mybir.ActivationFunctionType.Exp
    MULT = mybir.AluOpType.mult

    nc = bacc.Bacc("TRN2", target_bir_lowering=False, debug=False,
                   num_devices=NCORES)

    xr = nc.dram_tensor("xr", [C, N], BF16, kind="ExternalInput").ap()
    yr = nc.dram_tensor("yr", [C, M], BF16, kind="ExternalInput").ap()
    wq_d = nc.dram_tensor("wq", [C, 128], BF16, kind="ExternalInput").ap()
    wk_d = nc.dram_tensor("wk", [C, 128], BF16, kind="ExternalInput").ap()
    wv_d = nc.dram_tensor("wv", [C, 128], BF16, kind="ExternalInput").ap()
    wp_d = nc.dram_tensor("wp", [128, C], BF16, kind="ExternalInput").ap()
    outT = nc.dram_tensor("outT", [C, N], F16, kind="ExternalOutput").ap()

    with tile.TileContext(nc) as tc, ExitStack() as ctx:
        sb = ctx.enter_context(tc.tile_pool(name="sb", bufs=1))
        ppool = ctx.enter_context(tc.tile_pool(name="ppool", bufs=6))
        npool = ctx.enter_context(tc.tile_pool(name="npool", bufs=4))
        apool = ctx.enter_context(tc.tile_pool(name="apool", bufs=2))
        spool = ctx.enter_context(tc.tile_pool(name="spool", bufs=2))
        opool = ctx.enter_context(tc.tile_pool(name="opool", bufs=2))
        psA = ctx.enter_context(tc.tile_pool(name="psA", bufs=3, space="PSUM"))
        psB = ctx.enter_context(tc.tile_pool(name="psB", bufs=2, space="PSUM"))

        # ---- constants ----
        ident = sb.tile([128, 128], BF16, tag="ident")
        make_identity(nc, ident)
        # v2 tiles: [m 128, 130] bf16; cols 64/129 stay 1.0 (denominator)
        v2 = [sb.tile([128, 130], BF16, tag=f"v2_{m}", name=f"v2_{m}")
              for m in range(16)]
        for m in range(16):
            nc.gpsimd.memset(v2[m], 1.0)
        # warm the exp table while DMAs stream
        warm = sb.tile([1, 32], F32, tag="warm")
        nc.scalar.activation(warm, ident[0:1, 0:32], EXP)
        # warm the PE clock so early projections run fast
        psw = psB.tile([128, 512], F32, tag="acc", name="psw")
        for _ in range(8):
            nc.tensor.matmul(psw[:, 0:128], ident, ident, start=True, stop=True)
        warm2 = sb.tile([128, 128], F32, tag="warm2")
        nc.vector.tensor_copy(warm2, psw[:, 0:128])

        wq_sb = sb.tile([128, 4, 128], BF16, tag="wq_sb")
        wk_sb = sb.tile([128, 4, 128], BF16, tag="wk_sb")
        wv_sb = sb.tile([128, 4, 128], BF16, tag="wv_sb")
        wp_sb = sb.tile([128, C], BF16, tag="wp_sb")

        y_sb = sb.tile([128, 4, M], BF16, tag="y_sb")
        x_sb = sb.tile([128, 4, N], BF16, tag="x_sb")

        def load_j(dst, src, j):
            js = slice(j * 512, (j + 1) * 512)
            nc.sync.dma_start(
                out=dst[:, :, js],
                in_=src[:, js].rearrange("(kc p) m -> p kc m", p=128))

        # DMA order = consumption order; one DMA per j-chunk.  The first
        # exp is gated by wk+wq+y_j0+x_j0, so those four go first.
        nc.sync.dma_start(
            out=wk_sb, in_=wk_d.rearrange("(kc p) m -> p kc m", p=128))
        nc.sync.dma_start(
            out=wq_sb, in_=wq_d.rearrange("(kc p) m -> p kc m", p=128))
        load_j(y_sb, yr, 0)
        load_j(x_sb, xr, 0)
        nc.sync.dma_start(
            out=wv_sb, in_=wv_d.rearrange("(kc p) m -> p kc m", p=128))
        load_j(y_sb, yr, 1)
        load_j(y_sb, yr, 2)
        load_j(y_sb, yr, 3)
        nc.sync.dma_start(out=wp_sb, in_=wp_d)
        load_j(x_sb, xr, 1)
        load_j(x_sb, xr, 2)
        load_j(x_sb, xr, 3)

        kT = sb.tile([128, M], F32R, tag="kT")
        qT = sb.tile([128, N], F32R, tag="qT")

        def proj(dst, w_sb, src, j, name, use_act=False):
            ps = psA.tile([128, 512], F32, tag="blk", name=name)
            for kc in range(4):
                nc.tensor.matmul(ps, w_sb[:, kc, :],
                                 src[:, kc, j * 512:(j + 1) * 512],
                                 start=(kc == 0), stop=(kc == 3))
            if use_act:
                nc.scalar.copy(dst[:, j * 512:(j + 1) * 512], ps)
            else:
                nc.vector.tensor_copy(dst[:, j * 512:(j + 1) * 512], ps)

        def v2_proj(m):
            ps = psA.tile([128, 128], F32, tag="blk", name=f"psv{m}")
            for kc in range(4):
                nc.tensor.matmul(ps, y_sb[:, kc, m * 128:(m + 1) * 128],
                                 wv_sb[:, kc, :],
                                 start=(kc == 0), stop=(kc == 3))
            nc.vector.tensor_copy(v2[m][:, 0:64], ps[:, 0:64])
            nc.vector.tensor_copy(v2[m][:, 65:129], ps[:, 64:128])

        # ---- prologue: only the j0 projections gate the first exp ----
        proj(kT, wk_sb, y_sb, 0, "psk0", use_act=True)
        proj(qT, wq_sb, x_sb, 0, "psq0", use_act=True)

        # fill task groups woven between score blocks (chunk -> per-m lists);
        # v2_proj(m) must land a few steps before attnout m (lag 3-5)
        fills = {
            0: [[lambda: v2_proj(0), lambda: v2_proj(1)],
                [lambda: v2_proj(2), lambda: v2_proj(3)],
                [lambda: proj(kT, wk_sb, y_sb, 1, "psk1")],
                [lambda: v2_proj(4), lambda: v2_proj(5)],
                [lambda: v2_proj(6), lambda: v2_proj(7)],
                [lambda: proj(kT, wk_sb, y_sb, 2, "psk2")],
                [lambda: v2_proj(8), lambda: v2_proj(9)],
                [lambda: v2_proj(10), lambda: v2_proj(11)],
                [lambda: proj(kT, wk_sb, y_sb, 3, "psk3")],
                [lambda: v2_proj(12), lambda: v2_proj(13)],
                [lambda: v2_proj(14), lambda: v2_proj(15)],
                [lambda: proj(qT, wq_sb, x_sb, 1, "psq1")]],
            1: [[lambda: proj(qT, wq_sb, x_sb, 2, "psq2")]],
            2: [[lambda: proj(qT, wq_sb, x_sb, 3, "psq3")]],
            3: [],
        }

        # ---- attention main loop ----
        from collections import deque
        aq = deque()         # (m, P, accA, accB) awaiting attnout
        drain = None         # [stage, chunk, state...] of the pending drain

        def emit_attnout(pm, pP, paccA, paccB):
            # pm==0/nb==0 is the first matmul into each fresh acc bank: its
            # start=True marks the whole bank pending-zero; later groups'
            # first writes then zero-init via the per-byte pending path.
            for nb in range(4):
                for h, acc in ((0, paccA), (1, paccB)):
                    nc.tensor.matmul(
                        acc[:, nb * 65:(nb + 1) * 65],
                        pP[:, h * 512 + nb * 128: h * 512 + (nb + 1) * 128],
                        v2[pm][:, h * 65: h * 65 + 65],
                        start=(pm == 0 and nb == 0),
                        stop=(pm == 15 and nb == 3 and h == 1),
                        skip_group_check=True)

        def emit_norm(q, qaccA, qaccB, use_act=False):
            # batched strided reciprocal of the 4 denominator columns per
            # bank, then 8 per-partition-scalar multiplies -> bf16
            nrm = npool.tile([128, 512], BF16, tag="nrm", name=f"nrm{q}")
            rds = []
            for h, acc in ((0, qaccA), (1, qaccB)):
                rd = spool.tile([128, 4], F32, tag=f"rd{h}", name=f"rd{q}_{h}")
                nc.vector.reciprocal(rd, acc[:, 64:261:65])
                rds.append(rd)
            for nb in range(4):
                for h, acc in ((0, qaccA), (1, qaccB)):
                    dst = nrm[:, nb * 128 + h * 64: nb * 128 + (h + 1) * 64]
                    src = acc[:, nb * 65: nb * 65 + 64]
                    if use_act and h == 1:
                        nc.scalar.mul(dst, src, rds[h][:, nb:nb + 1])
                    else:
                        nc.vector.tensor_scalar(dst, src,
                                                rds[h][:, nb:nb + 1], None,
                                                op0=MULT)
            return nrm

        def emit_transposes(q, nrm):
            # 4 transposes share one PSUM slot; one bf16 2x copy out
            tp = psA.tile([128, 512], BF16, tag="blk", name=f"tp{q}")
            for nb in range(4):
                nc.tensor.transpose(
                    tp[:, nb * 128:(nb + 1) * 128],
                    nrm[:, nb * 128:(nb + 1) * 128], ident)
            at = apool.tile([128, 512], BF16, tag="attT", name=f"attT{q}")
            nc.vector.tensor_copy(at, tp)
            return at

        def emit_outproj(q, at, half, so, use_act=False):
            # two output-channel blocks share one PSUM slot -> f16 halves
            po = psA.tile([128, 1024], F32, tag="blk", name=f"po{q}_{half}")
            for i in range(2):
                cb = half * 2 + i
                for nb in range(4):
                    nc.tensor.matmul(
                        po[:, i * 512 + nb * 128: i * 512 + (nb + 1) * 128],
                        wp_sb[:, cb * 128:(cb + 1) * 128],
                        at[:, nb * 128:(nb + 1) * 128],
                        start=(nb == 0), stop=(nb == 3 and i == 1),
                        skip_group_check=True)
            if use_act:
                nc.scalar.copy(so[:, half * 1024:(half + 1) * 1024], po)
            else:
                nc.vector.tensor_copy(so[:, half * 1024:(half + 1) * 1024], po)

        def emit_outdma(q, so):
            nc.sync.dma_start(
                out=outT[:, q * 512:(q + 1) * 512].rearrange(
                    "(cb p) n -> p cb n", p=128),
                in_=so.rearrange("p (cb n) -> p cb n", n=512))

        for n in range(4):
            ns = slice(n * 512, (n + 1) * 512)
            accA = psB.tile([128, 512], F32, tag="acc", name=f"accA{n}")
            accB = psB.tile([128, 512], F32, tag="acc", name=f"accB{n}")
            for m in range(16):
                ms = slice(m * 128, (m + 1) * 128)
                blk = psA.tile([128, 1024], F32, tag="blk",
                               name=f"blk{n}_{m}")
                nc.tensor.matmul(blk[:, 0:512], kT[0:64, ms], qT[0:64, ns],
                                 start=True, stop=True, tile_position=(0, 0))
                nc.tensor.matmul(blk[:, 512:1024], kT[64:128, ms],
                                 qT[64:128, ns],
                                 start=True, stop=True, tile_position=(64, 0))
                P = ppool.tile([128, 1024], BF16, tag="p", name=f"p{n}_{m}")
                nc.scalar.activation(P, blk, EXP)
                # lag attnout 3-5 steps behind exp so the previous chunk's
                # normalize (reading the acc banks this chunk recycles) is
                # done before the PE's in-order queue reaches attnout m0
                aq.append((m, P, accA, accB))
                thresh = 4 if m in (3, 4) else 3
                while len(aq) > thresh:
                    e = aq.popleft()
                    emit_attnout(*e)
                    if e[0] == 15:
                        # chunk n-1 fully accumulated: kick its normalize
                        drain = [0, n - 1, emit_norm(n - 1, e[2], e[3]), None]
                if m >= 1 and fills[n]:
                    for task in fills[n].pop(0):
                        task()
                if drain is not None:
                    stage, dq, dstate, dso = drain
                    if stage == 0 and m >= 5:
                        drain = [1, dq, emit_transposes(dq, dstate),
                                 opool.tile([128, 2048], F16, tag="so",
                                            name=f"so{dq}")]
                    elif stage == 1:
                        emit_outproj(dq, dstate, 0, dso)
                        drain[0] = 2
                    elif stage == 2:
                        emit_outproj(dq, dstate, 1, dso)
                        emit_outdma(dq, dso)
                        drain = None

        # ---- epilogue: drain the final chunk (ScalarE is idle now, so it
        # takes half the normalize and one outproj copy) ----
        last = None
        while aq:
            last = aq.popleft()
            emit_attnout(*last)
        nrm = emit_norm(3, last[2], last[3], use_act=True)
        at = emit_transposes(3, nrm)
        so = opool.tile([128, 2048], F16, tag="so", name="so3")
        emit_outproj(3, at, 0, so, use_act=True)
        emit_outproj(3, at, 1, so)
        emit_outdma(3, so)

    nc.compile()
    return nc


def _get_program():
    global _NC
    if _NC is None:
        _NC = _build_program()
    return _NC


def make_in_maps(inputs):
    import ml_dtypes
    bf16 = ml_dtypes.bfloat16

    x = np.asarray(inputs["x"], np.float32)
    y = np.asarray(inputs["y"], np.float32)
    Wq = np.asarray(inputs["Wq"], np.float32)
    Wkv = np.asarray(inputs["Wkv"], np.float32)
    lw = np.asarray(inputs["lw"], np.float32)
    Wp = np.asarray(inputs["Wp"], np.float32)

    d = np.arange(HD)
    xr = [np.ascontiguousarray(x[b].astype(bf16)) for b in range(B)]
    yr = [np.ascontiguousarray(y[b].astype(bf16)) for b in range(B)]
    in_maps = []
    for core in range(NCORES):
        b = core // 4
        h0 = (core % 4) * 2
        ch = np.concatenate([h * HD + d for h in (h0, h0 + 1)])  # channels
        colsK = np.concatenate([h * 2 * HD + 2 * d for h in (h0, h0 + 1)])
        wq_c = Wq[:, ch] * np.float32(SCALE)
        wk_c = Wkv[:, colsK]
        wv_c = Wkv[:, colsK + 1] * (1.0 + lw[ch])[None, :]
        in_maps.append({
            "xr": xr[b],
            "yr": yr[b],
            "wq": np.ascontiguousarray(wq_c.astype(bf16)),
            "wk": np.ascontiguousarray(wk_c.astype(bf16)),
            "wv": np.ascontiguousarray(wv_c.astype(bf16)),
            "wp": np.ascontiguousarray(Wp[ch, :].astype(bf16)),
        })
    return in_maps


def assemble_output(results, inputs):
    lb = np.asarray(inputs["lb"], np.float32)
    Wp = np.asarray(inputs["Wp"], np.float32)
    bp = np.asarray(inputs["bp"], np.float32)
    bias = (bp + lb @ Wp).astype(np.float32)
    parts = [np.asarray(results[c]["outT"], dtype=np.float32)
             for c in range(NCORES)]
    out = np.stack([parts[0] + parts[1] + parts[2] + parts[3],
                    parts[4] + parts[5] + parts[6] + parts[7]])
    out += bias[None, :, None]
    return out.astype(np.float32)


def kernel(x, y, Wq, Wkv, lw, lb, Wp, bp):
    global LAST_RUN
    from concourse.bass_utils import run_bass_kernel_spmd

    inputs = dict(x=x, y=y, Wq=Wq, Wkv=Wkv, lw=lw, lb=lb, Wp=Wp, bp=bp)
    nc = _get_program()
    in_maps = make_in_maps(inputs)
    LAST_RUN = run_bass_kernel_spmd(nc, in_maps, list(range(NCORES)))
    return assemble_output(LAST_RUN.results, inputs)


# revision 15
# speedup vs baseline: 1.1809x; 1.0667x over previous
"""Trainium2 Bass kernel for nn_CrossAttention (B=2, C=512, N=M=2048, H=8).

Sharding: batch*heads = 16 (b,h) pairs across 8 cores, 2 heads per core.
Cores 0-3 handle batch 0 (heads 0..7 in pairs), cores 4-7 batch 1.

The kernel is ScalarE-exp-bound (softmax needs 65536 exp rows/core at
0.833ns — a ~55us engine floor no other engine can take), so PE work is
restructured to fit under it:
  qT[d,n] = (Wq*SCALE).T @ x_b   (bf16)                           8192c
  kT[d,m] = Wk.T @ y_b           (bf16, f32r in SBUF)             8192c
  v2[m,d] = y_b.T-slices @ (Wv*(1+lw))  direct [m,d] layout,      8192c
            bf16, no PE transposes; ones cols give the denominator
  S^T[m,n] = kT.T-slices @ qT   (K=64 pairs tile_position-packed) 65536c
  P = exp(S^T) -> bf16          (ScalarE, 64x [128,1024] blocks)
  att[n, d|den] += P_slice.T @ v2[m]   n-major: 128 out partitions,
            65-row bf16 matmuls (half the m-major cost)           33280c
  att_nrm[n,d2] = att * recip(den)     (DVE, per-partition scalar)
  attT[d2,n] = transpose(att_nrm)      (PE, bf16 identity)         2048c
  outT_partial[c,n] = Wp_rows.T @ attT  (bf16) -> f16 partials     8192c

The depthwise conv (ksize=1) folds into Wv scaling + a host-side output
bias (bias' = bp + lb @ Wp, exact because softmax rows sum to 1).
Host sums the 4 per-batch f16 partials in f32 and adds bias'.

PSUM: psA 3x[128,1024] ring (scores/exp; also proj, v2, transposes and
outproj transients) = 6 banks; psB 2x[128,512] = 2 banks holding the
8 per-chunk attnout accumulators (4x65 cols per bank; only the first
matmul into a bank uses start=True — the bank-wide pending-zero then
zero-initializes each co-located accumulation group on first touch).

Chunk q's drain (normalize/transpose/outproj/DMA) is woven into chunk
q+1's first m-steps so the PE's in-order queue and the psA ring never
stall the score stream that feeds ScalarE.
"""

import os
import sys
import numpy as np
from contextlib import ExitStack

for _p in ("/root/.axon_site", "/root/.axon_site/_ro/trn_rl_repo",
           "/root/.axon_site/_ro/pypackages", "/opt/trn_rl_repo"):
    if os.path.isdir(_p) and _p not in sys.path:
        sys.path.append(_p)

B, C, N, M, H = 2, 512, 2048, 2048, 8
HD = C // H
SCALE = HD ** -0.5
NCORES = 8

_NC = None
LAST_RUN = None


def _build_program():
    from concourse import bacc
    import concourse.tile as tile
    import concourse.mybir as mybir
    from concourse.masks import make_identity

    F32 = mybir.dt.float32
    F32R = mybir.dt.float32r
    BF16 = mybir.dt.bfloat16
    F16 = mybir.dt.float16
    EXP = mybir.ActivationFunctionType.Exp
    MULT = mybir.AluOpType.mult

    nc = bacc.Bacc("TRN2", target_bir_lowering=False, debug=False,
                   num_devices=NCORES)

    xr = nc.dram_tensor("xr", [C, N], BF16, kind="ExternalInput").ap()
    yr = nc.dram_tensor("yr", [C, M], BF16, kind="ExternalInput").ap()
    wq_d = nc.dram_tensor("wq", [C, 128], BF16, kind="ExternalInput").ap()
    wk_d = nc.dram_tensor("wk", [C, 128], BF16, kind="ExternalInput").ap()
    wv_d = nc.dram_tensor("wv", [C, 128], BF16, kind="ExternalInput").ap()
    wp_d = nc.dram_tensor("wp", [128, C], BF16, kind="ExternalInput").ap()
    outT = nc.dram_tensor("outT", [C, N], F16, kind="ExternalOutput").ap()

    with tile.TileContext(nc) as tc, ExitStack() as ctx:
        sb = ctx.enter_context(tc.tile_pool(name="sb", bufs=1))
        ppool = ctx.enter_context(tc.tile_pool(name="ppool", bufs=6))
        npool = ctx.enter_context(tc.tile_pool(name="npool", bufs=4))
        apool = ctx.enter_context(tc.tile_pool(name="apool", bufs=2))
        spool = ctx.enter_context(tc.tile_pool(name="spool", bufs=2))
        opool = ctx.enter_context(tc.tile_pool(name="opool", bufs=2))
        psA = ctx.enter_context(tc.tile_pool(name="psA", bufs=3, space="PSUM"))
        psB = ctx.enter_context(tc.tile_pool(name="psB", bufs=2, space="PSUM"))

        # ---- constants ----
        ident = sb.tile([128, 128], BF16, tag="ident")
        make_identity(nc, ident)
        # v2 group tiles: [m 128, mi 4, 130] bf16; cols 64/129 stay 1.0
        # (the softmax-denominator ones columns)
        v2g = [sb.tile([128, 4, 130], BF16, tag=f"v2g_{g}", name=f"v2g_{g}")
               for g in range(4)]
        for g in range(4):
            nc.gpsimd.memset(v2g[g], 1.0)

        def v2s(m, lo, hi):
            return v2g[m // 4][:, m % 4, lo:hi]
        # warm the exp table while DMAs stream
        warm = sb.tile([1, 32], F32, tag="warm")
        nc.scalar.activation(warm, ident[0:1, 0:32], EXP)
        # warm the PE clock so early projections run fast
        psw = psB.tile([128, 512], F32, tag="acc", name="psw")
        for _ in range(8):
            nc.tensor.matmul(psw[:, 0:128], ident, ident, start=True, stop=True)
        warm2 = sb.tile([128, 128], F32, tag="warm2")
        nc.vector.tensor_copy(warm2, psw[:, 0:128])

        wq_sb = sb.tile([128, 4, 128], BF16, tag="wq_sb")
        wk_sb = sb.tile([128, 4, 128], BF16, tag="wk_sb")
        wv_sb = sb.tile([128, 4, 128], BF16, tag="wv_sb")
        wp_sb = sb.tile([128, C], BF16, tag="wp_sb")

        y_sb = sb.tile([128, 4, M], BF16, tag="y_sb")
        x_sb = sb.tile([128, 4, N], BF16, tag="x_sb")

        def load_j(dst, src, j, half=None):
            js = slice(j * 512, (j + 1) * 512)
            ks = slice(0, 4) if half is None else slice(half * 2, half * 2 + 2)
            cs = slice(ks.start * 128, ks.stop * 128)
            nc.sync.dma_start(
                out=dst[:, ks, js],
                in_=src[cs, js].rearrange("(kc p) m -> p kc m", p=128))

        # DMA order = consumption order; one DMA per j-chunk.  The first
        # exp is gated by wk+wq+y_j0+x_j0, so those four go first.
        nc.sync.dma_start(
            out=wk_sb, in_=wk_d.rearrange("(kc p) m -> p kc m", p=128))
        nc.sync.dma_start(
            out=wq_sb, in_=wq_d.rearrange("(kc p) m -> p kc m", p=128))
        load_j(y_sb, yr, 0, half=0)
        load_j(y_sb, yr, 0, half=1)
        load_j(x_sb, xr, 0, half=0)
        load_j(x_sb, xr, 0, half=1)
        nc.sync.dma_start(
            out=wv_sb, in_=wv_d.rearrange("(kc p) m -> p kc m", p=128))
        load_j(y_sb, yr, 1)
        load_j(y_sb, yr, 2)
        load_j(y_sb, yr, 3)
        nc.sync.dma_start(out=wp_sb, in_=wp_d)
        load_j(x_sb, xr, 1)
        load_j(x_sb, xr, 2)
        load_j(x_sb, xr, 3)

        kT = sb.tile([128, M], F32R, tag="kT")
        qT = sb.tile([128, N], F32R, tag="qT")

        def proj(dst, w_sb, src, j, name, use_act=False):
            ps = psA.tile([128, 512], F32, tag="blk", name=name)
            for kc in range(4):
                nc.tensor.matmul(ps, w_sb[:, kc, :],
                                 src[:, kc, j * 512:(j + 1) * 512],
                                 start=(kc == 0), stop=(kc == 3))
            if use_act:
                nc.scalar.copy(dst[:, j * 512:(j + 1) * 512], ps)
            else:
                nc.vector.tensor_copy(dst[:, j * 512:(j + 1) * 512], ps)

        def v2_proj4(g):
            # four m-blocks share one PSUM bank; only the very first matmul
            # uses start=True (bank-wide pending-zero inits the rest)
            ps = psA.tile([128, 4, 128], F32, tag="blk", name=f"psv{g}")
            for mi in range(4):
                m = g * 4 + mi
                for kc in range(4):
                    nc.tensor.matmul(ps[:, mi, :],
                                     y_sb[:, kc, m * 128:(m + 1) * 128],
                                     wv_sb[:, kc, :],
                                     start=(mi == 0 and kc == 0),
                                     stop=(mi == 3 and kc == 3),
                                     skip_group_check=True)
            nc.vector.tensor_copy(v2g[g][:, :, 0:64], ps[:, :, 0:64])
            nc.vector.tensor_copy(v2g[g][:, :, 65:129], ps[:, :, 64:128])

        # ---- prologue: only the j0 projections gate the first exp ----
        proj(kT, wk_sb, y_sb, 0, "psk0", use_act=True)
        proj(qT, wq_sb, x_sb, 0, "psq0", use_act=True)

        # fill task groups woven between score blocks (chunk -> per-m lists);
        # v2_proj4(g) must land a few steps before attnout of m=4g (lag 3-5)
        fills = {
            0: [[lambda: v2_proj4(0)],
                [lambda: proj(kT, wk_sb, y_sb, 1, "psk1")],
                [lambda: v2_proj4(1)],
                [lambda: v2_proj4(2)],
                [lambda: proj(kT, wk_sb, y_sb, 2, "psk2")],
                [lambda: v2_proj4(3)],
                [],
                [],
                [lambda: proj(kT, wk_sb, y_sb, 3, "psk3")],
                [],
                [],
                [lambda: proj(qT, wq_sb, x_sb, 1, "psq1")]],
            1: [[lambda: proj(qT, wq_sb, x_sb, 2, "psq2")]],
            2: [[lambda: proj(qT, wq_sb, x_sb, 3, "psq3")]],
            3: [],
        }

        # ---- attention main loop ----
        from collections import deque
        aq = deque()         # (m, P, accA, accB) awaiting attnout
        drain = None         # [stage, chunk, state...] of the pending drain

        def emit_attnout(pm, pP, paccA, paccB):
            # pm==0/nb==0 is the first matmul into each fresh acc bank: its
            # start=True marks the whole bank pending-zero; later groups'
            # first writes then zero-init via the per-byte pending path.
            for nb in range(4):
                for h, acc in ((0, paccA), (1, paccB)):
                    nc.tensor.matmul(
                        acc[:, nb * 65:(nb + 1) * 65],
                        pP[:, h * 512 + nb * 128: h * 512 + (nb + 1) * 128],
                        v2s(pm, h * 65, h * 65 + 65),
                        start=(pm == 0 and nb == 0),
                        stop=(pm == 15 and nb == 3 and h == 1),
                        skip_group_check=True)

        def emit_norm(q, qaccA, qaccB, use_act=False):
            # batched strided reciprocal of the 4 denominator columns per
            # bank, then 8 per-partition-scalar multiplies -> bf16.
            # Per-nb nrm tiles keep the 8 writers dependency-free.
            nrms = []
            rds = []
            for h, acc in ((0, qaccA), (1, qaccB)):
                rd = spool.tile([128, 4], F32, tag=f"rd{h}", name=f"rd{q}_{h}")
                nc.vector.reciprocal(rd, acc[:, 64:261:65])
                rds.append(rd)
            for nb in range(4):
                nrm = npool.tile([128, 128], BF16, tag=f"nrm{nb}",
                                 name=f"nrm{q}_{nb}")
                for h, acc in ((0, qaccA), (1, qaccB)):
                    dst = nrm[:, h * 64:(h + 1) * 64]
                    src = acc[:, nb * 65: nb * 65 + 64]
                    if use_act and h == 1:
                        nc.scalar.mul(dst, src, rds[h][:, nb:nb + 1])
                    else:
                        nc.vector.tensor_scalar(dst, src,
                                                rds[h][:, nb:nb + 1], None,
                                                op0=MULT)
                nrms.append(nrm)
            return nrms

        def emit_transposes(q, nrms):
            # 4 transposes share one PSUM slot; one bf16 2x copy out
            tp = psA.tile([128, 512], BF16, tag="blk", name=f"tp{q}")
            for nb in range(4):
                nc.tensor.transpose(
                    tp[:, nb * 128:(nb + 1) * 128], nrms[nb], ident)
            at = apool.tile([128, 512], BF16, tag="attT", name=f"attT{q}")
            nc.vector.tensor_copy(at, tp)
            return at

        def emit_outproj(q, at, half, so, use_act=False):
            # two output-channel blocks share one PSUM slot -> f16 halves
            po = psA.tile([128, 1024], F32, tag="blk", name=f"po{q}_{half}")
            for i in range(2):
                cb = half * 2 + i
                for nb in range(4):
                    nc.tensor.matmul(
                        po[:, i * 512 + nb * 128: i * 512 + (nb + 1) * 128],
                        wp_sb[:, cb * 128:(cb + 1) * 128],
                        at[:, nb * 128:(nb + 1) * 128],
                        start=(nb == 0), stop=(nb == 3 and i == 1),
                        skip_group_check=True)
            if use_act:
                nc.scalar.copy(so[:, half * 1024:(half + 1) * 1024], po)
            else:
                nc.vector.tensor_copy(so[:, half * 1024:(half + 1) * 1024], po)

        def emit_outdma(q, so):
            nc.sync.dma_start(
                out=outT[:, q * 512:(q + 1) * 512].rearrange(
                    "(cb p) n -> p cb n", p=128),
                in_=so.rearrange("p (cb n) -> p cb n", n=512))

        for n in range(4):
            ns = slice(n * 512, (n + 1) * 512)
            accA = psB.tile([128, 512], F32, tag="acc", name=f"accA{n}")
            accB = psB.tile([128, 512], F32, tag="acc", name=f"accB{n}")
            for m in range(16):
                ms = slice(m * 128, (m + 1) * 128)
                blk = psA.tile([128, 1024], F32, tag="blk",
                               name=f"blk{n}_{m}")
                nc.tensor.matmul(blk[:, 0:512], kT[0:64, ms], qT[0:64, ns],
                                 start=True, stop=True, tile_position=(0, 0))
                nc.tensor.matmul(blk[:, 512:1024], kT[64:128, ms],
                                 qT[64:128, ns],
                                 start=True, stop=True, tile_position=(64, 0))
                P = ppool.tile([128, 1024], BF16, tag="p", name=f"p{n}_{m}")
                nc.scalar.activation(P, blk, EXP)
                # lag attnout 3-5 steps behind exp so the previous chunk's
                # normalize (reading the acc banks this chunk recycles) is
                # done before the PE's in-order queue reaches attnout m0
                aq.append((m, P, accA, accB))
                thresh = 4 if m in (3, 4) else 3
                while len(aq) > thresh:
                    e = aq.popleft()
                    emit_attnout(*e)
                    if e[0] == 15:
                        # chunk n-1 fully accumulated: kick its normalize
                        drain = [0, n - 1, emit_norm(n - 1, e[2], e[3]), None]
                if m >= 1 and fills[n]:
                    for task in fills[n].pop(0):
                        task()
                if drain is not None:
                    stage, dq, dstate, dso = drain
                    if stage == 0 and m >= 5:
                        drain = [1, dq, emit_transposes(dq, dstate),
                                 opool.tile([128, 2048], F16, tag="so",
                                            name=f"so{dq}")]
                    elif stage == 1:
                        emit_outproj(dq, dstate, 0, dso)
                        drain[0] = 2
                    elif stage == 2:
                        emit_outproj(dq, dstate, 1, dso)
                        emit_outdma(dq, dso)
                        drain = None

        # ---- epilogue: drain the final chunk, pipelined per nb-pair
        # (ScalarE is idle now: it takes the h1 normalize + cb0/1 copies)
        last = None
        while aq:
            last = aq.popleft()
            emit_attnout(*last)
        nrms = emit_norm(3, last[2], last[3], use_act=True)
        tp = psA.tile([128, 512], BF16, tag="blk", name="tp3")
        at = apool.tile([128, 512], BF16, tag="attT", name="attT3")
        po = [psA.tile([128, 1024], F32, tag="blk", name=f"po3_{ph}")
              for ph in (0, 1)]
        so = opool.tile([128, 2048], F16, tag="so", name="so3")
        sor = so.rearrange("p (cb n) -> p cb n", n=512)
        for nbp in (0, 1):
            for nb in (2 * nbp, 2 * nbp + 1):
                nc.tensor.transpose(tp[:, nb * 128:(nb + 1) * 128],
                                    nrms[nb], ident)
            nc.vector.tensor_copy(at[:, nbp * 256:(nbp + 1) * 256],
                                  tp[:, nbp * 256:(nbp + 1) * 256])
            for ph in (0, 1):
                for i in (0, 1):
                    cb = 2 * ph + i
                    for nb in (2 * nbp, 2 * nbp + 1):
                        nc.tensor.matmul(
                            po[ph][:, i * 512 + nb * 128:
                                   i * 512 + (nb + 1) * 128],
                            wp_sb[:, cb * 128:(cb + 1) * 128],
                            at[:, nb * 128:(nb + 1) * 128],
                            start=(nb == 0), stop=(nb == 3),
                            skip_group_check=True)
            for ph in (0, 1):
                psrc = po[ph].rearrange("p (i n) -> p i n", n=512)[
                    :, :, nbp * 256:(nbp + 1) * 256]
                pdst = sor[:, 2 * ph:2 * ph + 2, nbp * 256:(nbp + 1) * 256]
                if ph == 0:
                    nc.scalar.copy(pdst, psrc)
                else:
                    nc.vector.tensor_copy(pdst, psrc)
            nc.sync.dma_start(
                out=outT[:, 1536 + nbp * 256: 1536 + (nbp + 1) * 256
                         ].rearrange("(cb p) n -> p cb n", p=128),
                in_=sor[:, :, nbp * 256:(nbp + 1) * 256])

    nc.compile()
    return nc


def _get_program():
    global _NC
    if _NC is None:
        _NC = _build_program()
    return _NC


def make_in_maps(inputs):
    import ml_dtypes
    bf16 = ml_dtypes.bfloat16

    x = np.asarray(inputs["x"], np.float32)
    y = np.asarray(inputs["y"], np.float32)
    Wq = np.asarray(inputs["Wq"], np.float32)
    Wkv = np.asarray(inputs["Wkv"], np.float32)
    lw = np.asarray(inputs["lw"], np.float32)
    Wp = np.asarray(inputs["Wp"], np.float32)

    d = np.arange(HD)
    xr = [np.ascontiguousarray(x[b].astype(bf16)) for b in range(B)]
    yr = [np.ascontiguousarray(y[b].astype(bf16)) for b in range(B)]
    in_maps = []
    for core in range(NCORES):
        b = core // 4
        h0 = (core % 4) * 2
        ch = np.concatenate([h * HD + d for h in (h0, h0 + 1)])  # channels
        colsK = np.concatenate([h * 2 * HD + 2 * d for h in (h0, h0 + 1)])
        wq_c = Wq[:, ch] * np.float32(SCALE)
        wk_c = Wkv[:, colsK]
        wv_c = Wkv[:, colsK + 1] * (1.0 + lw[ch])[None, :]
        in_maps.append({
            "xr": xr[b],
            "yr": yr[b],
            "wq": np.ascontiguousarray(wq_c.astype(bf16)),
            "wk": np.ascontiguousarray(wk_c.astype(bf16)),
            "wv": np.ascontiguousarray(wv_c.astype(bf16)),
            "wp": np.ascontiguousarray(Wp[ch, :].astype(bf16)),
        })
    return in_maps


def assemble_output(results, inputs):
    lb = np.asarray(inputs["lb"], np.float32)
    Wp = np.asarray(inputs["Wp"], np.float32)
    bp = np.asarray(inputs["bp"], np.float32)
    bias = (bp + lb @ Wp).astype(np.float32)
    parts = [np.asarray(results[c]["outT"], dtype=np.float32)
             for c in range(NCORES)]
    out = np.stack([parts[0] + parts[1] + parts[2] + parts[3],
                    parts[4] + parts[5] + parts[6] + parts[7]])
    out += bias[None, :, None]
    return out.astype(np.float32)


def kernel(x, y, Wq, Wkv, lw, lb, Wp, bp):
    global LAST_RUN
    from concourse.bass_utils import run_bass_kernel_spmd

    inputs = dict(x=x, y=y, Wq=Wq, Wkv=Wkv, lw=lw, lb=lb, Wp=Wp, bp=bp)
    nc = _get_program()
    in_maps = make_in_maps(inputs)
    LAST_RUN = run_bass_kernel_spmd(nc, in_maps, list(range(NCORES)))
    return assemble_output(LAST_RUN.results, inputs)


# revision 16
# speedup vs baseline: 1.1892x; 1.0070x over previous
"""Trainium2 Bass kernel for nn_CrossAttention (B=2, C=512, N=M=2048, H=8).

Sharding: batch*heads = 16 (b,h) pairs across 8 cores, 2 heads per core.
Cores 0-3 handle batch 0 (heads 0..7 in pairs), cores 4-7 batch 1.

The kernel is ScalarE-exp-bound (softmax needs 65536 exp rows/core at
0.833ns — a ~55us engine floor no other engine can take), so PE work is
restructured to fit under it:
  qT[d,n] = (Wq*SCALE).T @ x_b   (bf16)                           8192c
  kT[d,m] = Wk.T @ y_b           (bf16, f32r in SBUF)             8192c
  v2[m,d] = y_b.T-slices @ (Wv*(1+lw))  direct [m,d] layout,      8192c
            bf16, no PE transposes; ones cols give the denominator
  S^T[m,n] = kT.T-slices @ qT   (K=64 pairs tile_position-packed) 65536c
  P = exp(S^T) -> bf16          (ScalarE, 64x [128,1024] blocks)
  att[n, d|den] += P_slice.T @ v2[m]   n-major: 128 out partitions,
            65-row bf16 matmuls (half the m-major cost)           33280c
  att_nrm[n,d2] = att * recip(den)     (DVE, per-partition scalar)
  attT[d2,n] = transpose(att_nrm)      (PE, bf16 identity)         2048c
  outT_partial[c,n] = Wp_rows.T @ attT  (bf16) -> f16 partials     8192c

The depthwise conv (ksize=1) folds into Wv scaling + a host-side output
bias (bias' = bp + lb @ Wp, exact because softmax rows sum to 1).
Host sums the 4 per-batch f16 partials in f32 and adds bias'.

PSUM: psA 3x[128,1024] ring (scores/exp; also proj, v2, transposes and
outproj transients) = 6 banks; psB 2x[128,512] = 2 banks holding the
8 per-chunk attnout accumulators (4x65 cols per bank; only the first
matmul into a bank uses start=True — the bank-wide pending-zero then
zero-initializes each co-located accumulation group on first touch).

Chunk q's drain (normalize/transpose/outproj/DMA) is woven into chunk
q+1's first m-steps so the PE's in-order queue and the psA ring never
stall the score stream that feeds ScalarE.
"""

import os
import sys
import numpy as np
from contextlib import ExitStack

for _p in ("/root/.axon_site", "/root/.axon_site/_ro/trn_rl_repo",
           "/root/.axon_site/_ro/pypackages", "/opt/trn_rl_repo"):
    if os.path.isdir(_p) and _p not in sys.path:
        sys.path.append(_p)

B, C, N, M, H = 2, 512, 2048, 2048, 8
HD = C // H
SCALE = HD ** -0.5
NCORES = 8

_NC = None
LAST_RUN = None


def _build_program():
    from concourse import bacc
    import concourse.tile as tile
    import concourse.mybir as mybir
    from concourse.masks import make_identity

    F32 = mybir.dt.float32
    F32R = mybir.dt.float32r
    BF16 = mybir.dt.bfloat16
    F16 = mybir.dt.float16
    EXP = mybir.ActivationFunctionType.Exp
    MULT = mybir.AluOpType.mult

    nc = bacc.Bacc("TRN2", target_bir_lowering=False, debug=False,
                   num_devices=NCORES)

    xr = nc.dram_tensor("xr", [C, N], BF16, kind="ExternalInput").ap()
    yr = nc.dram_tensor("yr", [C, M], BF16, kind="ExternalInput").ap()
    wq_d = nc.dram_tensor("wq", [C, 128], BF16, kind="ExternalInput").ap()
    wk_d = nc.dram_tensor("wk", [C, 128], BF16, kind="ExternalInput").ap()
    wv_d = nc.dram_tensor("wv", [C, 128], BF16, kind="ExternalInput").ap()
    wp_d = nc.dram_tensor("wp", [128, C], BF16, kind="ExternalInput").ap()
    outT = nc.dram_tensor("outT", [C, N], F16, kind="ExternalOutput").ap()

    with tile.TileContext(nc) as tc, ExitStack() as ctx:
        sb = ctx.enter_context(tc.tile_pool(name="sb", bufs=1))
        ppool = ctx.enter_context(tc.tile_pool(name="ppool", bufs=6))
        npool = ctx.enter_context(tc.tile_pool(name="npool", bufs=4))
        apool = ctx.enter_context(tc.tile_pool(name="apool", bufs=2))
        spool = ctx.enter_context(tc.tile_pool(name="spool", bufs=2))
        opool = ctx.enter_context(tc.tile_pool(name="opool", bufs=2))
        psA = ctx.enter_context(tc.tile_pool(name="psA", bufs=3, space="PSUM"))
        psB = ctx.enter_context(tc.tile_pool(name="psB", bufs=2, space="PSUM"))

        # ---- constants ----
        ident = sb.tile([128, 128], BF16, tag="ident")
        make_identity(nc, ident)
        # v2 group tiles: [m 128, mi 4, 130] bf16; cols 64/129 stay 1.0
        # (the softmax-denominator ones columns)
        v2g = [sb.tile([128, 4, 130], BF16, tag=f"v2g_{g}", name=f"v2g_{g}")
               for g in range(4)]
        for g in range(4):
            nc.gpsimd.memset(v2g[g], 1.0)

        def v2s(m, lo, hi):
            return v2g[m // 4][:, m % 4, lo:hi]
        # warm the exp table while DMAs stream
        warm = sb.tile([1, 32], F32, tag="warm")
        nc.scalar.activation(warm, ident[0:1, 0:32], EXP)
        # warm the PE clock so early projections run fast
        psw = psB.tile([128, 512], F32, tag="acc", name="psw")
        for _ in range(8):
            nc.tensor.matmul(psw[:, 0:128], ident, ident, start=True, stop=True)
        warm2 = sb.tile([128, 128], F32, tag="warm2")
        nc.vector.tensor_copy(warm2, psw[:, 0:128])

        wq_sb = sb.tile([128, 4, 128], BF16, tag="wq_sb")
        wk_sb = sb.tile([128, 4, 128], BF16, tag="wk_sb")
        wv_sb = sb.tile([128, 4, 128], BF16, tag="wv_sb")
        wp_sb = sb.tile([128, C], BF16, tag="wp_sb")

        y_sb = sb.tile([128, 4, M], BF16, tag="y_sb")
        x_sb = sb.tile([128, 4, N], BF16, tag="x_sb")

        def load_j(dst, src, j, half=None):
            js = slice(j * 512, (j + 1) * 512)
            ks = slice(0, 4) if half is None else slice(half * 2, half * 2 + 2)
            cs = slice(ks.start * 128, ks.stop * 128)
            nc.sync.dma_start(
                out=dst[:, ks, js],
                in_=src[cs, js].rearrange("(kc p) m -> p kc m", p=128))

        # DMA order = consumption order; one DMA per j-chunk.  The first
        # exp is gated by wk+wq+y_j0+x_j0, so those four go first.
        nc.sync.dma_start(
            out=wk_sb, in_=wk_d.rearrange("(kc p) m -> p kc m", p=128))
        nc.sync.dma_start(
            out=wq_sb, in_=wq_d.rearrange("(kc p) m -> p kc m", p=128))
        load_j(y_sb, yr, 0, half=0)
        load_j(y_sb, yr, 0, half=1)
        load_j(x_sb, xr, 0, half=0)
        load_j(x_sb, xr, 0, half=1)
        nc.sync.dma_start(
            out=wv_sb, in_=wv_d.rearrange("(kc p) m -> p kc m", p=128))
        load_j(y_sb, yr, 1)
        load_j(y_sb, yr, 2)
        load_j(y_sb, yr, 3)
        nc.sync.dma_start(out=wp_sb, in_=wp_d)
        load_j(x_sb, xr, 1)
        load_j(x_sb, xr, 2)
        load_j(x_sb, xr, 3)

        kT = sb.tile([128, M], F32R, tag="kT")
        qT = sb.tile([128, N], F32R, tag="qT")

        def proj(dst, w_sb, src, j, name, use_act=False):
            ps = psA.tile([128, 512], F32, tag="blk", name=name)
            for kc in range(4):
                nc.tensor.matmul(ps, w_sb[:, kc, :],
                                 src[:, kc, j * 512:(j + 1) * 512],
                                 start=(kc == 0), stop=(kc == 3))
            if use_act:
                nc.scalar.copy(dst[:, j * 512:(j + 1) * 512], ps)
            else:
                nc.vector.tensor_copy(dst[:, j * 512:(j + 1) * 512], ps)

        def v2_proj4(g):
            # four m-blocks share one PSUM bank; only the very first matmul
            # uses start=True (bank-wide pending-zero inits the rest)
            ps = psA.tile([128, 4, 128], F32, tag="blk", name=f"psv{g}")
            for mi in range(4):
                m = g * 4 + mi
                for kc in range(4):
                    nc.tensor.matmul(ps[:, mi, :],
                                     y_sb[:, kc, m * 128:(m + 1) * 128],
                                     wv_sb[:, kc, :],
                                     start=(mi == 0 and kc == 0),
                                     stop=(mi == 3 and kc == 3),
                                     skip_group_check=True)
            nc.vector.tensor_copy(v2g[g][:, :, 0:64], ps[:, :, 0:64])
            nc.vector.tensor_copy(v2g[g][:, :, 65:129], ps[:, :, 64:128])

        # ---- prologue: only the j0 projections gate the first exp ----
        proj(kT, wk_sb, y_sb, 0, "psk0", use_act=True)
        proj(qT, wq_sb, x_sb, 0, "psq0", use_act=True)

        # fill task groups woven between score blocks (chunk -> per-m lists);
        # v2_proj4(g) must land a few steps before attnout of m=4g (lag 3-5)
        fills = {
            0: [[lambda: v2_proj4(0)],
                [lambda: proj(kT, wk_sb, y_sb, 1, "psk1")],
                [lambda: v2_proj4(1)],
                [lambda: v2_proj4(2)],
                [lambda: proj(kT, wk_sb, y_sb, 2, "psk2")],
                [lambda: v2_proj4(3)],
                [],
                [],
                [lambda: proj(kT, wk_sb, y_sb, 3, "psk3")],
                [],
                [],
                [lambda: proj(qT, wq_sb, x_sb, 1, "psq1")]],
            1: [[lambda: proj(qT, wq_sb, x_sb, 2, "psq2")]],
            2: [[lambda: proj(qT, wq_sb, x_sb, 3, "psq3")]],
            3: [],
        }

        # ---- attention main loop ----
        from collections import deque
        aq = deque()         # (m, P, accA, accB) awaiting attnout
        drain = None         # [stage, chunk, state...] of the pending drain

        def emit_attnout(pm, pP, paccA, paccB):
            # pm==0/nb==0 is the first matmul into each fresh acc bank: its
            # start=True marks the whole bank pending-zero; later groups'
            # first writes then zero-init via the per-byte pending path.
            for nb in range(4):
                for h, acc in ((0, paccA), (1, paccB)):
                    nc.tensor.matmul(
                        acc[:, nb * 65:(nb + 1) * 65],
                        pP[:, h * 512 + nb * 128: h * 512 + (nb + 1) * 128],
                        v2s(pm, h * 65, h * 65 + 65),
                        start=(pm == 0 and nb == 0),
                        stop=(pm == 15 and nb == 3 and h == 1),
                        skip_group_check=True)

        def emit_norm(q, qaccA, qaccB):
            # batched strided reciprocal of the 4 denominator columns per
            # bank, then ONE broadcast tensor_tensor per bank: the [128,4]
            # reciprocals broadcast (stride-0) along the 64 d-columns
            nrm = npool.tile([128, 4, 128], BF16, tag="nrm", name=f"nrm{q}")
            for h, acc in ((0, qaccA), (1, qaccB)):
                rd = spool.tile([128, 4], F32, tag=f"rd{h}", name=f"rd{q}_{h}")
                nc.vector.reciprocal(rd, acc[:, 64:261:65])
                av = acc[:, 0:260].rearrange("p (nb c) -> p nb c", c=65)
                nc.vector.tensor_tensor(
                    nrm[:, :, h * 64:(h + 1) * 64], av[:, :, 0:64],
                    rd.to_broadcast([128, 4, 64]), op=MULT)
            return nrm

        def emit_transposes(q, nrm):
            # 4 transposes share one PSUM slot; one bf16 2x copy out
            tp = psA.tile([128, 512], BF16, tag="blk", name=f"tp{q}")
            for nb in range(4):
                nc.tensor.transpose(
                    tp[:, nb * 128:(nb + 1) * 128], nrm[:, nb, :], ident)
            at = apool.tile([128, 512], BF16, tag="attT", name=f"attT{q}")
            nc.vector.tensor_copy(at, tp)
            return at

        def emit_outproj(q, at, half, so, use_act=False):
            # two output-channel blocks share one PSUM slot -> f16 halves
            po = psA.tile([128, 1024], F32, tag="blk", name=f"po{q}_{half}")
            for i in range(2):
                cb = half * 2 + i
                for nb in range(4):
                    nc.tensor.matmul(
                        po[:, i * 512 + nb * 128: i * 512 + (nb + 1) * 128],
                        wp_sb[:, cb * 128:(cb + 1) * 128],
                        at[:, nb * 128:(nb + 1) * 128],
                        start=(nb == 0), stop=(nb == 3 and i == 1),
                        skip_group_check=True)
            if use_act:
                nc.scalar.copy(so[:, half * 1024:(half + 1) * 1024], po)
            else:
                nc.vector.tensor_copy(so[:, half * 1024:(half + 1) * 1024], po)

        def emit_outdma(q, so):
            nc.sync.dma_start(
                out=outT[:, q * 512:(q + 1) * 512].rearrange(
                    "(cb p) n -> p cb n", p=128),
                in_=so.rearrange("p (cb n) -> p cb n", n=512))

        for n in range(4):
            ns = slice(n * 512, (n + 1) * 512)
            accA = psB.tile([128, 512], F32, tag="acc", name=f"accA{n}")
            accB = psB.tile([128, 512], F32, tag="acc", name=f"accB{n}")
            for m in range(16):
                ms = slice(m * 128, (m + 1) * 128)
                blk = psA.tile([128, 1024], F32, tag="blk",
                               name=f"blk{n}_{m}")
                nc.tensor.matmul(blk[:, 0:512], kT[0:64, ms], qT[0:64, ns],
                                 start=True, stop=True, tile_position=(0, 0))
                nc.tensor.matmul(blk[:, 512:1024], kT[64:128, ms],
                                 qT[64:128, ns],
                                 start=True, stop=True, tile_position=(64, 0))
                P = ppool.tile([128, 1024], BF16, tag="p", name=f"p{n}_{m}")
                nc.scalar.activation(P, blk, EXP)
                # lag attnout 3-5 steps behind exp so the previous chunk's
                # normalize (reading the acc banks this chunk recycles) is
                # done before the PE's in-order queue reaches attnout m0
                aq.append((m, P, accA, accB))
                thresh = 4 if m in (3, 4) else 3
                while len(aq) > thresh:
                    e = aq.popleft()
                    emit_attnout(*e)
                    if e[0] == 15:
                        # chunk n-1 fully accumulated: kick its normalize
                        drain = [0, n - 1, emit_norm(n - 1, e[2], e[3]), None]
                if m >= 1 and fills[n]:
                    for task in fills[n].pop(0):
                        task()
                if drain is not None:
                    stage, dq, dstate, dso = drain
                    if stage == 0 and m >= 5:
                        drain = [1, dq, emit_transposes(dq, dstate),
                                 opool.tile([128, 2048], F16, tag="so",
                                            name=f"so{dq}")]
                    elif stage == 1:
                        emit_outproj(dq, dstate, 0, dso)
                        drain[0] = 2
                    elif stage == 2:
                        emit_outproj(dq, dstate, 1, dso)
                        emit_outdma(dq, dso)
                        drain = None

        # ---- epilogue: drain the final chunk, pipelined per nb-pair
        # (ScalarE is idle now: it takes the h1 normalize + cb0/1 copies)
        last = None
        while aq:
            last = aq.popleft()
            emit_attnout(*last)
        nrm3 = emit_norm(3, last[2], last[3])
        tp = psA.tile([128, 512], BF16, tag="blk", name="tp3")
        at = apool.tile([128, 512], BF16, tag="attT", name="attT3")
        po = [psA.tile([128, 1024], F32, tag="blk", name=f"po3_{ph}")
              for ph in (0, 1)]
        so = opool.tile([128, 2048], F16, tag="so", name="so3")
        sor = so.rearrange("p (cb n) -> p cb n", n=512)
        for nb in range(4):
            nc.tensor.transpose(tp[:, nb * 128:(nb + 1) * 128],
                                nrm3[:, nb, :], ident)
        nc.vector.tensor_copy(at[:, 0:256], tp[:, 0:256])
        nc.vector.tensor_copy(at[:, 256:512], tp[:, 256:512])
        for nbp in (0, 1):
            for ph in (0, 1):
                for i in (0, 1):
                    cb = 2 * ph + i
                    for nb in (2 * nbp, 2 * nbp + 1):
                        nc.tensor.matmul(
                            po[ph][:, i * 512 + nb * 128:
                                   i * 512 + (nb + 1) * 128],
                            wp_sb[:, cb * 128:(cb + 1) * 128],
                            at[:, nb * 128:(nb + 1) * 128],
                            start=(nb == 0), stop=(nb == 3),
                            skip_group_check=True)
            for ph in (0, 1):
                psrc = po[ph].rearrange("p (i n) -> p i n", n=512)[
                    :, :, nbp * 256:(nbp + 1) * 256]
                pdst = sor[:, 2 * ph:2 * ph + 2, nbp * 256:(nbp + 1) * 256]
                if ph == 0:
                    nc.scalar.copy(pdst, psrc)
                else:
                    nc.vector.tensor_copy(pdst, psrc)
            nc.sync.dma_start(
                out=outT[:, 1536 + nbp * 256: 1536 + (nbp + 1) * 256
                         ].rearrange("(cb p) n -> p cb n", p=128),
                in_=sor[:, :, nbp * 256:(nbp + 1) * 256])

    nc.compile()
    return nc


def _get_program():
    global _NC
    if _NC is None:
        _NC = _build_program()
    return _NC


def make_in_maps(inputs):
    import ml_dtypes
    bf16 = ml_dtypes.bfloat16

    x = np.asarray(inputs["x"], np.float32)
    y = np.asarray(inputs["y"], np.float32)
    Wq = np.asarray(inputs["Wq"], np.float32)
    Wkv = np.asarray(inputs["Wkv"], np.float32)
    lw = np.asarray(inputs["lw"], np.float32)
    Wp = np.asarray(inputs["Wp"], np.float32)

    d = np.arange(HD)
    xr = [np.ascontiguousarray(x[b].astype(bf16)) for b in range(B)]
    yr = [np.ascontiguousarray(y[b].astype(bf16)) for b in range(B)]
    in_maps = []
    for core in range(NCORES):
        b = core // 4
        h0 = (core % 4) * 2
        ch = np.concatenate([h * HD + d for h in (h0, h0 + 1)])  # channels
        colsK = np.concatenate([h * 2 * HD + 2 * d for h in (h0, h0 + 1)])
        wq_c = Wq[:, ch] * np.float32(SCALE)
        wk_c = Wkv[:, colsK]
        wv_c = Wkv[:, colsK + 1] * (1.0 + lw[ch])[None, :]
        in_maps.append({
            "xr": xr[b],
            "yr": yr[b],
            "wq": np.ascontiguousarray(wq_c.astype(bf16)),
            "wk": np.ascontiguousarray(wk_c.astype(bf16)),
            "wv": np.ascontiguousarray(wv_c.astype(bf16)),
            "wp": np.ascontiguousarray(Wp[ch, :].astype(bf16)),
        })
    return in_maps


def assemble_output(results, inputs):
    lb = np.asarray(inputs["lb"], np.float32)
    Wp = np.asarray(inputs["Wp"], np.float32)
    bp = np.asarray(inputs["bp"], np.float32)
    bias = (bp + lb @ Wp).astype(np.float32)
    parts = [np.asarray(results[c]["outT"], dtype=np.float32)
             for c in range(NCORES)]
    out = np.stack([parts[0] + parts[1] + parts[2] + parts[3],
                    parts[4] + parts[5] + parts[6] + parts[7]])
    out += bias[None, :, None]
    return out.astype(np.float32)


def kernel(x, y, Wq, Wkv, lw, lb, Wp, bp):
    global LAST_RUN
    from concourse.bass_utils import run_bass_kernel_spmd

    inputs = dict(x=x, y=y, Wq=Wq, Wkv=Wkv, lw=lw, lb=lb, Wp=Wp, bp=bp)
    nc = _get_program()
    in_maps = make_in_maps(inputs)
    LAST_RUN = run_bass_kernel_spmd(nc, in_maps, list(range(NCORES)))
    return assemble_output(LAST_RUN.results, inputs)


# revision 17
# speedup vs baseline: 1.2007x; 1.0097x over previous
"""Trainium2 Bass kernel for nn_CrossAttention (B=2, C=512, N=M=2048, H=8).

Sharding: batch*heads = 16 (b,h) pairs across 8 cores, 2 heads per core.
Cores 0-3 handle batch 0 (heads 0..7 in pairs), cores 4-7 batch 1.

The kernel is ScalarE-exp-bound (softmax needs 65536 exp rows/core at
0.833ns — a ~55us engine floor no other engine can take), so PE work is
restructured to fit under it:
  qT[d,n] = (Wq*SCALE).T @ x_b   (bf16)                           8192c
  kT[d,m] = Wk.T @ y_b           (bf16, f32r in SBUF)             8192c
  v2[m,d] = y_b.T-slices @ (Wv*(1+lw))  direct [m,d] layout,      8192c
            bf16, no PE transposes; ones cols give the denominator
  S^T[m,n] = kT.T-slices @ qT   (K=64 pairs tile_position-packed) 65536c
  P = exp(S^T) -> bf16          (ScalarE, 64x [128,1024] blocks)
  att[n, d|den] += P_slice.T @ v2[m]   n-major: 128 out partitions,
            65-row bf16 matmuls (half the m-major cost)           33280c
  att_nrm[n,d2] = att * recip(den)     (DVE, per-partition scalar)
  attT[d2,n] = transpose(att_nrm)      (PE, bf16 identity)         2048c
  outT_partial[c,n] = Wp_rows.T @ attT  (bf16) -> f16 partials     8192c

The depthwise conv (ksize=1) folds into Wv scaling + a host-side output
bias (bias' = bp + lb @ Wp, exact because softmax rows sum to 1).
Host sums the 4 per-batch f16 partials in f32 and adds bias'.

PSUM: psA 3x[128,1024] ring (scores/exp; also proj, v2, transposes and
outproj transients) = 6 banks; psB 2x[128,512] = 2 banks holding the
8 per-chunk attnout accumulators (4x65 cols per bank; only the first
matmul into a bank uses start=True — the bank-wide pending-zero then
zero-initializes each co-located accumulation group on first touch).

Chunk q's drain (normalize/transpose/outproj/DMA) is woven into chunk
q+1's first m-steps so the PE's in-order queue and the psA ring never
stall the score stream that feeds ScalarE.
"""

import os
import sys
import numpy as np
from contextlib import ExitStack

for _p in ("/root/.axon_site", "/root/.axon_site/_ro/trn_rl_repo",
           "/root/.axon_site/_ro/pypackages", "/opt/trn_rl_repo"):
    if os.path.isdir(_p) and _p not in sys.path:
        sys.path.append(_p)

B, C, N, M, H = 2, 512, 2048, 2048, 8
HD = C // H
SCALE = HD ** -0.5
NCORES = 8

_NC = None
LAST_RUN = None


def _build_program():
    from concourse import bacc
    import concourse.tile as tile
    import concourse.mybir as mybir
    from concourse.masks import make_identity

    F32 = mybir.dt.float32
    F32R = mybir.dt.float32r
    BF16 = mybir.dt.bfloat16
    F16 = mybir.dt.float16
    EXP = mybir.ActivationFunctionType.Exp
    MULT = mybir.AluOpType.mult

    nc = bacc.Bacc("TRN2", target_bir_lowering=False, debug=False,
                   num_devices=NCORES)

    xr = nc.dram_tensor("xr", [C, N], BF16, kind="ExternalInput").ap()
    yr = nc.dram_tensor("yr", [C, M], BF16, kind="ExternalInput").ap()
    wq_d = nc.dram_tensor("wq", [C, 128], BF16, kind="ExternalInput").ap()
    wk_d = nc.dram_tensor("wk", [C, 128], BF16, kind="ExternalInput").ap()
    wv_d = nc.dram_tensor("wv", [C, 128], BF16, kind="ExternalInput").ap()
    wp_d = nc.dram_tensor("wp", [128, C], BF16, kind="ExternalInput").ap()
    outT = nc.dram_tensor("outT", [C, N], F16, kind="ExternalOutput").ap()

    with tile.TileContext(nc) as tc, ExitStack() as ctx:
        sb = ctx.enter_context(tc.tile_pool(name="sb", bufs=1))
        ppool = ctx.enter_context(tc.tile_pool(name="ppool", bufs=20))
        npool = ctx.enter_context(tc.tile_pool(name="npool", bufs=4))
        apool = ctx.enter_context(tc.tile_pool(name="apool", bufs=2))
        spool = ctx.enter_context(tc.tile_pool(name="spool", bufs=2))
        opool = ctx.enter_context(tc.tile_pool(name="opool", bufs=2))
        psA = ctx.enter_context(tc.tile_pool(name="psA", bufs=3, space="PSUM"))
        psB = ctx.enter_context(tc.tile_pool(name="psB", bufs=2, space="PSUM"))

        # ---- constants ----
        ident = sb.tile([128, 128], BF16, tag="ident")
        make_identity(nc, ident)
        # v2 group tiles: [m 128, mi 4, 130] bf16; cols 64/129 stay 1.0
        # (the softmax-denominator ones columns)
        v2g = [sb.tile([128, 4, 130], BF16, tag=f"v2g_{g}", name=f"v2g_{g}")
               for g in range(4)]
        for g in range(4):
            nc.gpsimd.memset(v2g[g], 1.0)

        def v2s(m, lo, hi):
            return v2g[m // 4][:, m % 4, lo:hi]
        # warm the exp table while DMAs stream
        warm = sb.tile([1, 32], F32, tag="warm")
        nc.scalar.activation(warm, ident[0:1, 0:32], EXP)
        # warm the PE clock so early projections run fast
        psw = psB.tile([128, 512], F32, tag="acc", name="psw")
        for _ in range(8):
            nc.tensor.matmul(psw[:, 0:128], ident, ident, start=True, stop=True)
        warm2 = sb.tile([128, 128], F32, tag="warm2")
        nc.vector.tensor_copy(warm2, psw[:, 0:128])

        wq_sb = sb.tile([128, 4, 128], BF16, tag="wq_sb")
        wk_sb = sb.tile([128, 4, 128], BF16, tag="wk_sb")
        wv_sb = sb.tile([128, 4, 128], BF16, tag="wv_sb")
        wp_sb = sb.tile([128, C], BF16, tag="wp_sb")

        y_sb = sb.tile([128, 4, M], BF16, tag="y_sb")
        x_sb = sb.tile([128, 4, N], BF16, tag="x_sb")

        def load_j(dst, src, j, half=None):
            js = slice(j * 512, (j + 1) * 512)
            ks = slice(0, 4) if half is None else slice(half * 2, half * 2 + 2)
            cs = slice(ks.start * 128, ks.stop * 128)
            nc.sync.dma_start(
                out=dst[:, ks, js],
                in_=src[cs, js].rearrange("(kc p) m -> p kc m", p=128))

        # DMA order = consumption order; one DMA per j-chunk.  The first
        # exp is gated by wk+wq+y_j0+x_j0, so those four go first.
        nc.sync.dma_start(
            out=wk_sb, in_=wk_d.rearrange("(kc p) m -> p kc m", p=128))
        nc.sync.dma_start(
            out=wq_sb, in_=wq_d.rearrange("(kc p) m -> p kc m", p=128))
        load_j(y_sb, yr, 0, half=0)
        load_j(y_sb, yr, 0, half=1)
        load_j(x_sb, xr, 0, half=0)
        load_j(x_sb, xr, 0, half=1)
        nc.sync.dma_start(
            out=wv_sb, in_=wv_d.rearrange("(kc p) m -> p kc m", p=128))
        load_j(y_sb, yr, 1)
        load_j(y_sb, yr, 2)
        load_j(y_sb, yr, 3)
        nc.sync.dma_start(out=wp_sb, in_=wp_d)
        load_j(x_sb, xr, 1)
        load_j(x_sb, xr, 2)
        load_j(x_sb, xr, 3)

        kT = sb.tile([128, M], F32R, tag="kT")
        qT = sb.tile([128, N], F32R, tag="qT")

        def proj(dst, w_sb, src, j, name, use_act=False):
            ps = psA.tile([128, 512], F32, tag="blk", name=name)
            for kc in range(4):
                nc.tensor.matmul(ps, w_sb[:, kc, :],
                                 src[:, kc, j * 512:(j + 1) * 512],
                                 start=(kc == 0), stop=(kc == 3))
            if use_act:
                nc.scalar.copy(dst[:, j * 512:(j + 1) * 512], ps)
            else:
                nc.vector.tensor_copy(dst[:, j * 512:(j + 1) * 512], ps)

        def v2_proj4(g):
            # four m-blocks share one PSUM bank; only the very first matmul
            # uses start=True (bank-wide pending-zero inits the rest)
            ps = psA.tile([128, 4, 128], F32, tag="blk", name=f"psv{g}")
            for mi in range(4):
                m = g * 4 + mi
                for kc in range(4):
                    nc.tensor.matmul(ps[:, mi, :],
                                     y_sb[:, kc, m * 128:(m + 1) * 128],
                                     wv_sb[:, kc, :],
                                     start=(mi == 0 and kc == 0),
                                     stop=(mi == 3 and kc == 3),
                                     skip_group_check=True)
            nc.vector.tensor_copy(v2g[g][:, :, 0:64], ps[:, :, 0:64])
            nc.vector.tensor_copy(v2g[g][:, :, 65:129], ps[:, :, 64:128])

        # ---- prologue: only the j0 projections gate the first exp ----
        proj(kT, wk_sb, y_sb, 0, "psk0", use_act=True)
        proj(qT, wq_sb, x_sb, 0, "psq0", use_act=True)

        # fill task groups woven between score blocks (chunk -> per-m lists);
        # v2_proj4(g) must land a few steps before attnout of m=4g (lag 3-5)
        fills = {
            0: [[lambda: v2_proj4(0)],
                [lambda: proj(kT, wk_sb, y_sb, 1, "psk1")],
                [lambda: v2_proj4(1)],
                [lambda: v2_proj4(2)],
                [lambda: proj(kT, wk_sb, y_sb, 2, "psk2")],
                [lambda: v2_proj4(3)],
                [],
                [],
                [lambda: proj(kT, wk_sb, y_sb, 3, "psk3")],
                [],
                [],
                [lambda: proj(qT, wq_sb, x_sb, 1, "psq1")]],
            1: [[lambda: proj(qT, wq_sb, x_sb, 2, "psq2")]],
            2: [[lambda: proj(qT, wq_sb, x_sb, 3, "psq3")]],
            3: [],
        }

        # ---- attention main loop ----
        from collections import deque
        aq = deque()         # (m, P, accA, accB) awaiting attnout
        drain = None         # [stage, chunk, state...] of the pending drain

        def emit_attnout(pm, pP, paccA, paccB):
            # pm==0/nb==0 is the first matmul into each fresh acc bank: its
            # start=True marks the whole bank pending-zero; later groups'
            # first writes then zero-init via the per-byte pending path.
            for h, acc in ((0, paccA), (1, paccB)):
                for nb in range(4):
                    nc.tensor.matmul(
                        acc[:, nb * 65:(nb + 1) * 65],
                        pP[:, h * 512 + nb * 128: h * 512 + (nb + 1) * 128],
                        v2s(pm, h * 65, h * 65 + 65),
                        start=(pm == 0 and nb == 0),
                        stop=(pm == 15 and nb == 3),
                        skip_group_check=True)

        def emit_norm(q, qaccA, qaccB):
            # batched strided reciprocal of the 4 denominator columns per
            # bank, then ONE broadcast tensor_tensor per bank: the [128,4]
            # reciprocals broadcast (stride-0) along the 64 d-columns
            nrm = npool.tile([128, 4, 128], BF16, tag="nrm", name=f"nrm{q}")
            for h, acc in ((0, qaccA), (1, qaccB)):
                rd = spool.tile([128, 4], F32, tag=f"rd{h}", name=f"rd{q}_{h}")
                nc.vector.reciprocal(rd, acc[:, 64:261:65])
                av = acc[:, 0:260].rearrange("p (nb c) -> p nb c", c=65)
                nc.vector.tensor_tensor(
                    nrm[:, :, h * 64:(h + 1) * 64], av[:, :, 0:64],
                    rd.to_broadcast([128, 4, 64]), op=MULT)
            return nrm

        def emit_transposes(q, nrm):
            # 4 transposes share one PSUM slot; one bf16 2x copy out
            tp = psA.tile([128, 512], BF16, tag="blk", name=f"tp{q}")
            for nb in range(4):
                nc.tensor.transpose(
                    tp[:, nb * 128:(nb + 1) * 128], nrm[:, nb, :], ident)
            at = apool.tile([128, 512], BF16, tag="attT", name=f"attT{q}")
            nc.vector.tensor_copy(at, tp)
            return at

        def emit_outproj(q, at, half, so, use_act=False):
            # two output-channel blocks share one PSUM slot -> f16 halves
            po = psA.tile([128, 1024], F32, tag="blk", name=f"po{q}_{half}")
            for i in range(2):
                cb = half * 2 + i
                for nb in range(4):
                    nc.tensor.matmul(
                        po[:, i * 512 + nb * 128: i * 512 + (nb + 1) * 128],
                        wp_sb[:, cb * 128:(cb + 1) * 128],
                        at[:, nb * 128:(nb + 1) * 128],
                        start=(nb == 0), stop=(nb == 3 and i == 1),
                        skip_group_check=True)
            if use_act:
                nc.scalar.copy(so[:, half * 1024:(half + 1) * 1024], po)
            else:
                nc.vector.tensor_copy(so[:, half * 1024:(half + 1) * 1024], po)

        def emit_outdma(q, so):
            nc.sync.dma_start(
                out=outT[:, q * 512:(q + 1) * 512].rearrange(
                    "(cb p) n -> p cb n", p=128),
                in_=so.rearrange("p (cb n) -> p cb n", n=512))

        for n in range(4):
            ns = slice(n * 512, (n + 1) * 512)
            accA = psB.tile([128, 512], F32, tag="acc", name=f"accA{n}")
            accB = psB.tile([128, 512], F32, tag="acc", name=f"accB{n}")
            for m in range(16):
                ms = slice(m * 128, (m + 1) * 128)
                blk = psA.tile([128, 1024], F32, tag="blk",
                               name=f"blk{n}_{m}")
                nc.tensor.matmul(blk[:, 0:512], kT[0:64, ms], qT[0:64, ns],
                                 start=True, stop=True, tile_position=(0, 0))
                nc.tensor.matmul(blk[:, 512:1024], kT[64:128, ms],
                                 qT[64:128, ns],
                                 start=True, stop=True, tile_position=(64, 0))
                P = ppool.tile([128, 1024], BF16, tag="p", name=f"p{n}_{m}")
                nc.scalar.activation(P, blk, EXP)
                # attnout scheduling: chunk 0 holds ALL its attnouts (the
                # chunk is PE-bound with the projection fills), chunk 1
                # drains the backlog at <=3/step in its PE slack; afterwards
                # a steady 3-5 step lag keeps the previous chunk's normalize
                # (reading the acc banks this chunk recycles) ahead of the
                # PE's in-order queue reaching attnout m0
                aq.append((m, P, accA, accB))
                if n == 0:
                    thresh, cap = 99, 0
                elif n == 1 and m < 6:
                    thresh, cap = 3, 3
                else:
                    thresh, cap = (4 if m in (3, 4) else 3), 2
                pops = 0
                while len(aq) > thresh and pops < cap:
                    pops += 1
                    e = aq.popleft()
                    emit_attnout(*e)
                    if e[0] == 15:
                        # chunk n-1 fully accumulated: kick its normalize
                        drain = [0, n - 1, emit_norm(n - 1, e[2], e[3]), None]
                if m >= 1 and fills[n]:
                    for task in fills[n].pop(0):
                        task()
                if drain is not None:
                    stage, dq, dstate, dso = drain
                    if stage == 0:
                        drain = [1, dq, dstate, dso]   # one-step gap for norm
                    elif stage == 1:
                        drain = [2, dq, emit_transposes(dq, dstate),
                                 opool.tile([128, 2048], F16, tag="so",
                                            name=f"so{dq}")]
                    elif stage == 2:
                        emit_outproj(dq, dstate, 0, dso)
                        drain[0] = 3
                    elif stage == 3:
                        emit_outproj(dq, dstate, 1, dso)
                        emit_outdma(dq, dso)
                        drain = None

        # ---- epilogue: drain the final chunk, pipelined per nb-pair
        # (ScalarE is idle now: it takes the h1 normalize + cb0/1 copies)
        last = None
        while aq:
            last = aq.popleft()
            emit_attnout(*last)
        nrm3 = emit_norm(3, last[2], last[3])
        tp = psA.tile([128, 512], BF16, tag="blk", name="tp3")
        at = apool.tile([128, 512], BF16, tag="attT", name="attT3")
        po = [psA.tile([128, 1024], F32, tag="blk", name=f"po3_{ph}")
              for ph in (0, 1)]
        so = opool.tile([128, 2048], F16, tag="so", name="so3")
        sor = so.rearrange("p (cb n) -> p cb n", n=512)
        for nb in range(4):
            nc.tensor.transpose(tp[:, nb * 128:(nb + 1) * 128],
                                nrm3[:, nb, :], ident)
        nc.vector.tensor_copy(at[:, 0:256], tp[:, 0:256])
        nc.vector.tensor_copy(at[:, 256:512], tp[:, 256:512])
        for nbp in (0, 1):
            for ph in (0, 1):
                for i in (0, 1):
                    cb = 2 * ph + i
                    for nb in (2 * nbp, 2 * nbp + 1):
                        nc.tensor.matmul(
                            po[ph][:, i * 512 + nb * 128:
                                   i * 512 + (nb + 1) * 128],
                            wp_sb[:, cb * 128:(cb + 1) * 128],
                            at[:, nb * 128:(nb + 1) * 128],
                            start=(nb == 0), stop=(nb == 3),
                            skip_group_check=True)
            for ph in (0, 1):
                psrc = po[ph].rearrange("p (i n) -> p i n", n=512)[
                    :, :, nbp * 256:(nbp + 1) * 256]
                pdst = sor[:, 2 * ph:2 * ph + 2, nbp * 256:(nbp + 1) * 256]
                if ph == 0:
                    nc.scalar.copy(pdst, psrc)
                else:
                    nc.vector.tensor_copy(pdst, psrc)
            nc.sync.dma_start(
                out=outT[:, 1536 + nbp * 256: 1536 + (nbp + 1) * 256
                         ].rearrange("(cb p) n -> p cb n", p=128),
                in_=sor[:, :, nbp * 256:(nbp + 1) * 256])

    nc.compile()
    return nc


def _get_program():
    global _NC
    if _NC is None:
        _NC = _build_program()
    return _NC


def make_in_maps(inputs):
    import ml_dtypes
    bf16 = ml_dtypes.bfloat16

    x = np.asarray(inputs["x"], np.float32)
    y = np.asarray(inputs["y"], np.float32)
    Wq = np.asarray(inputs["Wq"], np.float32)
    Wkv = np.asarray(inputs["Wkv"], np.float32)
    lw = np.asarray(inputs["lw"], np.float32)
    Wp = np.asarray(inputs["Wp"], np.float32)

    d = np.arange(HD)
    xr = [np.ascontiguousarray(x[b].astype(bf16)) for b in range(B)]
    yr = [np.ascontiguousarray(y[b].astype(bf16)) for b in range(B)]
    in_maps = []
    for core in range(NCORES):
        b = core // 4
        h0 = (core % 4) * 2
        ch = np.concatenate([h * HD + d for h in (h0, h0 + 1)])  # channels
        colsK = np.concatenate([h * 2 * HD + 2 * d for h in (h0, h0 + 1)])
        wq_c = Wq[:, ch] * np.float32(SCALE)
        wk_c = Wkv[:, colsK]
        wv_c = Wkv[:, colsK + 1] * (1.0 + lw[ch])[None, :]
        in_maps.append({
            "xr": xr[b],
            "yr": yr[b],
            "wq": np.ascontiguousarray(wq_c.astype(bf16)),
            "wk": np.ascontiguousarray(wk_c.astype(bf16)),
            "wv": np.ascontiguousarray(wv_c.astype(bf16)),
            "wp": np.ascontiguousarray(Wp[ch, :].astype(bf16)),
        })
    return in_maps


def assemble_output(results, inputs):
    lb = np.asarray(inputs["lb"], np.float32)
    Wp = np.asarray(inputs["Wp"], np.float32)
    bp = np.asarray(inputs["bp"], np.float32)
    bias = (bp + lb @ Wp).astype(np.float32)
    parts = [np.asarray(results[c]["outT"], dtype=np.float32)
             for c in range(NCORES)]
    out = np.stack([parts[0] + parts[1] + parts[2] + parts[3],
                    parts[4] + parts[5] + parts[6] + parts[7]])
    out += bias[None, :, None]
    return out.astype(np.float32)


def kernel(x, y, Wq, Wkv, lw, lb, Wp, bp):
    global LAST_RUN
    from concourse.bass_utils import run_bass_kernel_spmd

    inputs = dict(x=x, y=y, Wq=Wq, Wkv=Wkv, lw=lw, lb=lb, Wp=Wp, bp=bp)
    nc = _get_program()
    in_maps = make_in_maps(inputs)
    LAST_RUN = run_bass_kernel_spmd(nc, in_maps, list(range(NCORES)))
    return assemble_output(LAST_RUN.results, inputs)
